# revision 16
# baseline (speedup 1.0000x reference)
"""Trainium2 Bass kernel for nn_Attention: LN -> QKV -> per-head attention
(with k/v layernorm) -> output projection.

Sharding: 8 cores = 4 batches x 2 head-groups (8 heads each).  Each core
computes its batch's QKV restricted to its heads (no redundant matmul work),
runs attention for its 8 heads, and produces a partial output projection
(contraction over its 512 inner features).  The host sums the two partials
per batch and adds all bias terms.

Host-side weight folds (exact algebra, no device cost):
  - norm_g folded into w_qkv rows;  norm_b @ w_qkv becomes per-feature bias
    vectors cq/ck/cv added on device after each projection.
  - normk_g folded into w_q columns; normk_b dropped (a per-query constant
    added to all scores of a row cancels in softmax).
  - normv_g folded into w_out rows; normv_b term becomes a host-side output
    bias (sum of attention probs is exactly 1).
Softmax is computed without max subtraction (scores are O(1) for LN'd
activations; exp stays well inside fp32 range).  All matmuls run as fp32r.
"""

import os
import sys

import numpy as np

for _p in ("/opt/trn_rl_repo", "/root/.axon_site/_ro/trn_rl_repo"):
    if os.path.isdir(_p) and _p not in sys.path:
        sys.path.append(_p)

import concourse.bass as bass
import concourse.mybir as mybir
import concourse.tile as tile
from concourse.bass_utils import run_bass_kernel_spmd

FP32 = mybir.dt.float32
FP32R = mybir.dt.float32r
AF = mybir.ActivationFunctionType
OP = mybir.AluOpType

B = 4            # batch
S = 2048         # sequence length
C = 1024         # model dim
HEADS = 16
D = 64           # head dim
HG = 8           # heads per core
F = HG * D       # per-core q/k/v feature width (512)
O = 1024         # output dim
P = 128
EPS = 1e-5
N_CORES = 8

S_TILES = S // P          # 16
C_TILES = C // P          # 8
SB = 4                    # seq blocks
SBW = S // SB             # 512 cols per seq block
PAIRS = HG // 2           # 4 head pairs per core
Q4 = 4                    # query blocks of 512
SCALE = D ** -0.5


def _r(t):
    """fp32r view for matmul operands."""
    return t.bitcast(FP32R)


def _bcast_ap(ap_1d, parts):
    """[n] DRAM/SBUF AP -> [parts, n] with 0-step partition broadcast."""
    return bass.AP(tensor=ap_1d.tensor, offset=ap_1d.offset,
                   ap=[[0, parts]] + [list(x) for x in ap_1d.ap])


def _free_bcast(ap2d, n):
    """[p, m] AP -> [p, m, n] broadcasting each element n times along free."""
    return bass.AP(tensor=ap2d.tensor, offset=ap2d.offset,
                   ap=[list(x) for x in ap2d.ap] + [[0, n]])


def split_waits(nc, max_other=1):
    """walrus here rejects >1 sync-wait on TPB_CTRL (Drain) and may reject
    many on others; hoist extra waits onto preceding single-wait NoOps."""
    for f in nc.m.functions:
        for bb in f.blocks:
            new_insts = []
            for inst in bb.instructions:
                si = inst.sync_info
                limit = 1 if isinstance(
                    inst, (mybir.InstDrain, mybir.InstEventSemaphore,
                           mybir.InstNoOp)) else max_other
                if si and si.on_wait and len(si.on_wait) > limit:
                    waits = list(si.on_wait)
                    keep, extra = waits[-limit:], waits[:-limit]
                    for j, w in enumerate(extra):
                        nop = mybir.InstNoOp(
                            name=f"{inst.name}_wsplit_{j}", ins=[], outs=[])
                        nop.engine = inst.engine
                        nop.sync_info = mybir.SyncInfo(on_wait=[w], on_update=[])
                        new_insts.append(nop)
                    inst.sync_info = mybir.SyncInfo(
                        on_wait=keep, on_update=list(si.on_update))
                new_insts.append(inst)
            bb.instructions[:] = new_insts
    return nc


def build_nc(reps=None):
    from contextlib import ExitStack
    from concourse.masks import make_identity

    nc = bass.Bass()
    x_d = nc.declare_dram_parameter("x_s", [S, C], FP32, isOutput=False)
    wq_d = nc.declare_dram_parameter("wq", [C, F], FP32, isOutput=False)
    wk_d = nc.declare_dram_parameter("wk", [C, F], FP32, isOutput=False)
    wv_d = nc.declare_dram_parameter("wv", [C, F], FP32, isOutput=False)
    wo_d = nc.declare_dram_parameter("wo", [F, O], FP32, isOutput=False)
    cq_d = nc.declare_dram_parameter("cq", [F], FP32, isOutput=False)
    ck_d = nc.declare_dram_parameter("ck", [F], FP32, isOutput=False)
    cv_d = nc.declare_dram_parameter("cv", [F], FP32, isOutput=False)
    ones_d = nc.declare_dram_parameter("ones_c", [S_TILES * HG * (D + 1)], FP32, isOutput=False)
    out_d = nc.declare_dram_parameter("out_p", [S, O], FP32, isOutput=True)

    with tile.TileContext(nc) as tc, ExitStack() as ctx:
        if reps:
            ctx.enter_context(tc.For_i(0, reps, 1))
        singles = ctx.enter_context(tc.tile_pool(name="singles", bufs=1))
        acts = ctx.enter_context(tc.tile_pool(name="acts", bufs=1))

        # ---- persistent SBUF state ----
        ident = singles.tile([P, P], FP32)
        make_identity(nc, ident)
        wo_sb = singles.tile([P, F // P, O], FP32R)  # [128, 4, 1024]
        nc.sync.dma_start(out=wo_sb, in_=_r(wo_d.rearrange("(i p) o -> p i o", p=P)))
        cq_sb = singles.tile([P, F // P], FP32)      # [128, 4]
        nc.sync.dma_start(out=cq_sb, in_=cq_d.rearrange("(i p) -> p i", p=P))
        ck_bc = singles.tile([P, F], FP32)
        nc.sync.dma_start(out=ck_bc, in_=_bcast_ap(ck_d[:], P))
        cv_bc = singles.tile([P, F], FP32)
        nc.sync.dma_start(out=cv_bc, in_=_bcast_ap(cv_d[:], P))

        ones1 = singles.tile([1, D], FP32R)
        nc.sync.dma_start(out=ones1, in_=_r(ones_d[0:D][None, :]))
        eps_sb = singles.tile([P, 1], FP32)
        nc.vector.memset(eps_sb, EPS)
        v_sb = singles.tile([P, S_TILES, HG, D + 1], FP32R)  # ~33 KB/part
        nc.sync.dma_start(out=v_sb, in_=_r(_bcast_ap(ones_d[:], P)))
        qT_sb = acts.tile([P, PAIRS, S], FP32R)
        kT_sb = acts.tile([P, PAIRS, S], FP32R)

        # =========== phase 1+2: LN(x), transposes, Q/K/V projections =======
        with tc.tile_pool(name="p12", bufs=2) as p12, \
             tc.tile_pool(name="p12w", bufs=3) as p12w, \
             tc.tile_pool(name="p12s", bufs=4) as p12s, \
             tc.tile_pool(name="ps12", bufs=1, space="PSUM") as ps12:
            for sb in range(SB):
                xnT = p12.tile([P, C_TILES, SBW], FP32R, tag="xnT", bufs=1)
                # ---- x load + LN + transpose into xnT ----
                for t in range(SBW // P):
                    row0 = sb * SBW + t * P
                    x_t = p12.tile([P, C], FP32, tag="x")
                    nc.sync.dma_start(out=x_t, in_=x_d[row0:row0 + P, :])
                    xsum = p12s.tile([P, 1], FP32, tag="st1")
                    xn_t = p12.tile([P, C], FP32, tag="xn")
                    # mean via ACT copy+accum (out is scratch, overwritten below)
                    nc.scalar.activation(xn_t, x_t, AF.Copy, accum_out=xsum)
                    sq_scr = p12.tile([P, C], FP32, tag="sqscr", bufs=1)
                    nc.vector.tensor_mul(sq_scr, x_t, x_t)
                    xsqr = p12s.tile([P, 1], FP32, tag="st2")
                    nc.vector.tensor_reduce(xsqr, sq_scr,
                                            mybir.AxisListType.X, OP.add)
                    xsqm = p12s.tile([P, 1], FP32, tag="st2b")
                    nc.vector.tensor_scalar(xsqm, xsqr, 1.0 / C, None, OP.mult)
                    mu = p12s.tile([P, 1], FP32, tag="st3")
                    nc.vector.tensor_scalar(mu, xsum, 1.0 / C, None, OP.mult)
                    musq = p12s.tile([P, 1], FP32, tag="st4")
                    nc.vector.tensor_mul(musq, mu, mu)
                    var = p12s.tile([P, 1], FP32, tag="st5")
                    nc.vector.tensor_sub(var, xsqm, musq)
                    lnv = p12s.tile([P, 1], FP32, tag="st6")
                    nc.scalar.activation(lnv, var, AF.Ln, bias=eps_sb)
                    rstd = p12s.tile([P, 1], FP32, tag="st7")
                    nc.scalar.activation(rstd, lnv, AF.Exp, scale=-0.5)
                    nmr = p12s.tile([P, 1], FP32, tag="st8")
                    nc.vector.tensor_scalar(nmr, mu, rstd, -1.0, OP.mult, OP.mult)
                    # xn = x*rstd - mu*rstd  (one ACT pass)
                    nc.scalar.activation(xn_t, x_t, AF.Identity,
                                         bias=nmr, scale=rstd)
                    for ci in range(C_TILES):
                        tp = ps12.tile([P, P], FP32, tag="tp", bufs=2)
                        nc.tensor.transpose(tp, xn_t[:, ci * P:(ci + 1) * P],
                                            ident)
                        nc.vector.tensor_copy(xnT[:, ci, t * P:(t + 1) * P], tp)

                # ---- Q projection (transposed out): qT += wq.T @ xnT ----
                psq = [ps12.tile([P, SBW], FP32, tag="proj", bufs=4,
                                 name=f"psq{sb}_{_i}") for _i in range(F // P)]
                for ci in range(C_TILES):
                    w_t = p12w.tile([P, F], FP32R, tag="wstream")
                    nc.sync.dma_start(out=w_t,
                                      in_=_r(wq_d[ci * P:(ci + 1) * P, :]))
                    for fi in range(F // P):
                        nc.tensor.matmul(
                            psq[fi], w_t[:, fi * P:(fi + 1) * P],
                            xnT[:, ci, :],
                            start=(ci == 0), stop=(ci == C_TILES - 1))
                for fi in range(F // P):
                    nc.vector.tensor_scalar(
                        qT_sb[:, fi, sb * SBW:(sb + 1) * SBW], psq[fi],
                        cq_sb[:, fi:fi + 1], None, OP.add)

                # ---- K projection (natural out) + k-LN + transpose ----
                psk = [ps12.tile([P, F], FP32, tag="proj", bufs=4,
                                 name=f"psk{sb}_{_i}") for _i in range(SBW // P)]
                for ci in range(C_TILES):
                    w_t = p12w.tile([P, F], FP32R, tag="wstream")
                    nc.sync.dma_start(out=w_t,
                                      in_=_r(wk_d[ci * P:(ci + 1) * P, :]))
                    for st in range(SBW // P):
                        nc.tensor.matmul(
                            psk[st], xnT[:, ci, st * P:(st + 1) * P],
                            w_t, start=(ci == 0), stop=(ci == C_TILES - 1))
                for st in range(SBW // P):
                    kn = p12.tile([P, HG, D], FP32, tag="kn")
                    nc.vector.tensor_add(kn, psk[st].rearrange(
                        "p (h d) -> p h d", d=D), ck_bc.rearrange(
                        "p (h d) -> p h d", d=D))
                    _ln_hat(nc, tc, p12, p12s, kn, apply_out=kn, eps_sb=eps_sb)
                    # transpose per head pair into kT_sb
                    gst = sb * (SBW // P) + st
                    for pj in range(PAIRS):
                        tpk = ps12.tile([P, P], FP32, tag="tp", bufs=2)
                        nc.tensor.transpose(
                            tpk, kn[:, 2 * pj:2 * pj + 2, :], ident)
                        nc.vector.tensor_copy(
                            kT_sb[:, pj, gst * P:(gst + 1) * P], tpk)

                # ---- V projection (natural out) + v-LN into v_sb ----
                psv = [ps12.tile([P, F], FP32, tag="proj", bufs=4,
                                 name=f"psv{sb}_{_i}") for _i in range(SBW // P)]
                for ci in range(C_TILES):
                    w_t = p12w.tile([P, F], FP32R, tag="wstream")
                    nc.sync.dma_start(out=w_t,
                                      in_=_r(wv_d[ci * P:(ci + 1) * P, :]))
                    for st in range(SBW // P):
                        nc.tensor.matmul(
                            psv[st], xnT[:, ci, st * P:(st + 1) * P],
                            w_t, start=(ci == 0), stop=(ci == C_TILES - 1))
                for st in range(SBW // P):
                    vn = p12.tile([P, HG, D], FP32, tag="vn")
                    nc.vector.tensor_add(vn, psv[st].rearrange(
                        "p (h d) -> p h d", d=D), cv_bc.rearrange(
                        "p (h d) -> p h d", d=D))
                    gst = sb * (SBW // P) + st
                    _ln_hat(nc, tc, p12, p12s, vn,
                            apply_out=v_sb[:, gst, :, 0:D], eps_sb=eps_sb)

        # =========== phase 3: attention per head pair ======================
        with tc.tile_pool(name="p3", bufs=3) as p3, \
             tc.tile_pool(name="p3r", bufs=2) as p3r, \
             tc.tile_pool(name="ps3", bufs=1, space="PSUM") as ps3:
            attnT = acts.tile([P, PAIRS, S], FP32R)
            for pj in range(PAIRS):
                for q4 in range(Q4):
                    qs = q4 * SBW
                    poA = ps3.tile([D + 1, SBW], FP32, tag="po", bufs=2)
                    poB = ps3.tile([D + 1, SBW], FP32, tag="po", bufs=2)
                    for sk in range(S_TILES):
                        ks = sk * P
                        psA = ps3.tile([P, SBW], FP32, tag="ps", bufs=3)
                        psB = ps3.tile([P, SBW], FP32, tag="ps", bufs=3)
                        nc.tensor.matmul(psA, kT_sb[0:D, pj, ks:ks + P],
                                         qT_sb[0:D, pj, qs:qs + SBW])
                        nc.tensor.matmul(psB, kT_sb[D:P, pj, ks:ks + P],
                                         qT_sb[D:P, pj, qs:qs + SBW])
                        eA = p3.tile([P, SBW], FP32R, tag="eA")
                        eB = p3.tile([P, SBW], FP32R, tag="eB")
                        nc.scalar.activation(eA, psA, AF.Exp, scale=SCALE)
                        nc.scalar.activation(eB, psB, AF.Exp, scale=SCALE)
                        nc.tensor.matmul(poA, v_sb[:, sk, 2 * pj, :],
                                         eA, start=(sk == 0),
                                         stop=(sk == S_TILES - 1))
                        nc.tensor.matmul(poB, v_sb[:, sk, 2 * pj + 1, :],
                                         eB, start=(sk == 0),
                                         stop=(sk == S_TILES - 1))
                    for half, po in ((0, poA), (1, poB)):
                        rec = p3r.tile([1, SBW], FP32R, tag="rec")
                        with nc.allow_low_precision(reason="softmax recip to fp32r"):
                            nc.vector.reciprocal(rec, po[D:D + 1, :])
                        rbc_ps = ps3.tile([D, SBW], FP32, tag="rbc", bufs=2)
                        nc.tensor.matmul(rbc_ps, ones1, rec,
                                         start=True, stop=True)
                        rbc = p3r.tile([D, SBW], FP32, tag="rbc_sb", bufs=2)
                        nc.vector.tensor_copy(rbc, rbc_ps)
                        nc.vector.tensor_mul(
                            attnT[half * D:(half + 1) * D, pj, qs:qs + SBW],
                            po[0:D, :], rbc)

        # =========== phase 4: output projection ============================
        with tc.tile_pool(name="p4", bufs=3) as p4, \
             tc.tile_pool(name="ps4", bufs=2, space="PSUM") as ps4:
            for st in range(S_TILES):
                o_t = p4.tile([P, O], FP32, tag="ot")
                for oi in range(O // SBW):
                    pp = ps4.tile([P, SBW], FP32, tag="pp")
                    for ii in range(F // P):
                        nc.tensor.matmul(
                            pp, attnT[:, ii, st * P:(st + 1) * P],
                            wo_sb[:, ii, oi * SBW:(oi + 1) * SBW],
                            start=(ii == 0), stop=(ii == F // P - 1))
                    nc.vector.tensor_copy(o_t[:, oi * SBW:(oi + 1) * SBW], pp)
                nc.sync.dma_start(out=out_d[st * P:(st + 1) * P, :], in_=o_t)

    return nc


def _ln_hat(nc, tc, pool, spool, t_in, apply_out, eps_sb=None):
    """Per-head layernorm hat: (t - mean_d) * rsqrt(var_d + eps).
    t_in: [P, HG, D] sbuf tile; writes hat into apply_out ([P, HG, D] AP)."""
    sq = pool.tile([P, HG, D], FP32, tag="lnsq", bufs=1)
    nc.vector.tensor_mul(sq, t_in, t_in)
    sm = spool.tile([P, HG], FP32, tag="lns1")
    nc.vector.tensor_reduce(sm, t_in, mybir.AxisListType.X, OP.add)
    sqm = spool.tile([P, HG], FP32, tag="lns2")
    nc.vector.tensor_reduce(sqm, sq, mybir.AxisListType.X, OP.add)
    mu = spool.tile([P, HG], FP32, tag="lns3")
    nc.vector.tensor_scalar(mu, sm, 1.0 / D, None, OP.mult)
    esq = spool.tile([P, HG], FP32, tag="lns4")
    nc.vector.tensor_scalar(esq, sqm, 1.0 / D, None, OP.mult)
    musq = spool.tile([P, HG], FP32, tag="lns5")
    nc.vector.tensor_mul(musq, mu, mu)
    var = spool.tile([P, HG], FP32, tag="lns6")
    nc.vector.tensor_sub(var, esq, musq)
    lnv = spool.tile([P, HG], FP32, tag="lns7")
    nc.scalar.activation(lnv, var, AF.Ln, bias=eps_sb)
    rstd = spool.tile([P, HG], FP32, tag="lns8")
    nc.scalar.activation(rstd, lnv, AF.Exp, scale=-0.5)
    ctr = pool.tile([P, HG, D], FP32, tag="lnctr", bufs=1)
    nc.vector.tensor_tensor(ctr, t_in, _free_bcast(mu[:, :], D), OP.subtract)
    nc.vector.tensor_tensor(apply_out, ctr, _free_bcast(rstd[:, :], D),
                            OP.mult)


_NC_CACHE = None


def _get_nc():
    global _NC_CACHE
    if _NC_CACHE is None:
        nc = build_nc()
        split_waits(nc)
        _NC_CACHE = nc
    return _NC_CACHE


def prep_core_inputs(x, norm_g, norm_b, w_qkv, normk_g, normk_b,
                     normv_g, normv_b, w_out, b_out):
    """Host-side fold + shard.  Returns (in_maps, host_bias[core] (O,))."""
    x = np.asarray(x, np.float32)
    norm_g = np.asarray(norm_g, np.float32)
    norm_b = np.asarray(norm_b, np.float32)
    w_qkv = np.asarray(w_qkv, np.float32)
    normk_g = np.asarray(normk_g, np.float32)
    normv_g = np.asarray(normv_g, np.float32)
    normv_b = np.asarray(normv_b, np.float32)
    w_out = np.asarray(w_out, np.float32)

    INNER = HEADS * D
    wq_all, wk_all, wv_all = (w_qkv[:, 0:INNER], w_qkv[:, INNER:2 * INNER],
                              w_qkv[:, 2 * INNER:3 * INNER])
    gk_t = np.tile(normk_g, HG)          # [512] per head-group tiling
    gv_full = np.tile(normv_g, HEADS)
    bv_full = np.tile(normv_b, HEADS)

    in_maps, host_bias = [], []
    for core in range(N_CORES):
        b_idx, hg = divmod(core, 2)
        cols = slice(hg * F, (hg + 1) * F)
        wq = wq_all[:, cols]
        wk = wk_all[:, cols]
        wv = wv_all[:, cols]
        wo = w_out[cols, :]
        # fold norm_g into rows; normk_g into q columns
        wq_f = (norm_g[:, None] * wq) * gk_t[None, :]
        wk_f = norm_g[:, None] * wk
        wv_f = norm_g[:, None] * wv
        cq = (norm_b @ wq) * gk_t
        ck = norm_b @ wk
        cv = norm_b @ wv
        # fold normv_g into w_out rows; normv_b -> host bias
        wo_f = gv_full[cols][:, None] * wo
        host_bias.append(bv_full[cols] @ wo)
        in_maps.append({
            "x_s": np.ascontiguousarray(x[b_idx]),
            "wq": np.ascontiguousarray(wq_f),
            "wk": np.ascontiguousarray(wk_f),
            "wv": np.ascontiguousarray(wv_f),
            "wo": np.ascontiguousarray(wo_f),
            "cq": np.ascontiguousarray(cq),
            "ck": np.ascontiguousarray(ck),
            "cv": np.ascontiguousarray(cv),
            "ones_c": np.ones(S_TILES * HG * (D + 1), np.float32),
        })
    return in_maps, host_bias


def kernel(**inputs):
    nc = _get_nc()
    in_maps, host_bias = prep_core_inputs(**inputs)
    res = run_bass_kernel_spmd(nc, in_maps, list(range(N_CORES)))
    b_out = np.asarray(inputs["b_out"], np.float32)
    out = np.empty((B, S, O), np.float32)
    for b_idx in range(B):
        out[b_idx] = (res.results[2 * b_idx]["out_p"]
                      + res.results[2 * b_idx + 1]["out_p"]
                      + host_bias[2 * b_idx] + host_bias[2 * b_idx + 1]
                      + b_out)
    return out


if __name__ == "__main__":
    nc = build_nc()
    n = sum(len(bb.instructions) for f in nc.m.functions for bb in f.blocks)
    print("built ok,", n, "instructions")


# revision 21
# speedup vs baseline: 1.2934x; 1.2934x over previous
"""Trainium2 Bass kernel for nn_Attention: LN -> QKV -> per-head attention
(with k/v layernorm) -> output projection.

Sharding: 8 cores = 4 batches x 2 head-groups (8 heads each).  Each core
computes its batch's QKV restricted to its heads (no redundant matmul work),
runs attention for its 8 heads, and produces a partial output projection
(contraction over its 512 inner features).  The host sums the two partials
per batch and adds all bias terms.

Host-side weight folds (exact algebra, no device cost):
  - norm_g folded into w_qkv rows;  norm_b @ w_qkv becomes per-feature bias
    vectors cq/ck/cv added on device after each projection.
  - normk_g folded into w_q columns; normk_b dropped (a per-query constant
    added to all scores of a row cancels in softmax).
  - normv_g folded into w_out rows; normv_b term becomes a host-side output
    bias (sum of attention probs is exactly 1).
Softmax is computed without max subtraction (scores are O(1) for LN'd
activations; exp stays well inside fp32 range).  All matmuls run as fp32r.
"""

import os
import sys

import numpy as np

for _p in ("/opt/trn_rl_repo", "/root/.axon_site/_ro/trn_rl_repo"):
    if os.path.isdir(_p) and _p not in sys.path:
        sys.path.append(_p)

import concourse.bass as bass
import concourse.mybir as mybir
import concourse.tile as tile
from concourse.bass_utils import run_bass_kernel_spmd

FP32 = mybir.dt.float32
FP32R = mybir.dt.float32r
AF = mybir.ActivationFunctionType
OP = mybir.AluOpType

B = 4            # batch
S = 2048         # sequence length
C = 1024         # model dim
HEADS = 16
D = 64           # head dim
HG = 8           # heads per core
F = HG * D       # per-core q/k/v feature width (512)
O = 1024         # output dim
P = 128
EPS = 1e-5
N_CORES = 8

S_TILES = S // P          # 16
C_TILES = C // P          # 8
SB = 4                    # seq blocks
SBW = S // SB             # 512 cols per seq block
PAIRS = HG // 2           # 4 head pairs per core
Q4 = 4                    # query blocks of 512
SCALE = D ** -0.5


def _r(t):
    """fp32r view for matmul operands."""
    return t.bitcast(FP32R)


def _bcast_ap(ap_1d, parts):
    """[n] DRAM/SBUF AP -> [parts, n] with 0-step partition broadcast."""
    return bass.AP(tensor=ap_1d.tensor, offset=ap_1d.offset,
                   ap=[[0, parts]] + [list(x) for x in ap_1d.ap])


def _free_bcast(ap2d, n):
    """[p, m] AP -> [p, m, n] broadcasting each element n times along free."""
    return bass.AP(tensor=ap2d.tensor, offset=ap2d.offset,
                   ap=[list(x) for x in ap2d.ap] + [[0, n]])


def split_waits(nc, max_other=1):
    """walrus here rejects >1 sync-wait on TPB_CTRL (Drain) and may reject
    many on others; hoist extra waits onto preceding single-wait NoOps."""
    for f in nc.m.functions:
        for bb in f.blocks:
            new_insts = []
            for inst in bb.instructions:
                si = inst.sync_info
                limit = 1 if isinstance(
                    inst, (mybir.InstDrain, mybir.InstEventSemaphore,
                           mybir.InstNoOp)) else max_other
                if si and si.on_wait and len(si.on_wait) > limit:
                    waits = list(si.on_wait)
                    keep, extra = waits[-limit:], waits[:-limit]
                    for j, w in enumerate(extra):
                        nop = mybir.InstNoOp(
                            name=f"{inst.name}_wsplit_{j}", ins=[], outs=[])
                        nop.engine = inst.engine
                        nop.sync_info = mybir.SyncInfo(on_wait=[w], on_update=[])
                        new_insts.append(nop)
                    inst.sync_info = mybir.SyncInfo(
                        on_wait=keep, on_update=list(si.on_update))
                new_insts.append(inst)
            bb.instructions[:] = new_insts
    return nc


def build_nc(reps=None):
    from contextlib import ExitStack
    from concourse.masks import make_identity

    nc = bass.Bass()
    x_d = nc.declare_dram_parameter("x_s", [S, C], FP32, isOutput=False)
    wq_d = nc.declare_dram_parameter("wq", [C, F], FP32, isOutput=False)
    wk_d = nc.declare_dram_parameter("wk", [C, F], FP32, isOutput=False)
    wv_d = nc.declare_dram_parameter("wv", [C, F], FP32, isOutput=False)
    wo_d = nc.declare_dram_parameter("wo", [F, O], FP32, isOutput=False)
    cq_d = nc.declare_dram_parameter("cq", [F], FP32, isOutput=False)
    ck_d = nc.declare_dram_parameter("ck", [F], FP32, isOutput=False)
    cv_d = nc.declare_dram_parameter("cv", [F], FP32, isOutput=False)
    ones_d = nc.declare_dram_parameter("ones_c", [S_TILES * HG * (D + 1)], FP32, isOutput=False)
    out_d = nc.declare_dram_parameter("out_p", [S, O], FP32, isOutput=True)

    with tile.TileContext(nc) as tc, ExitStack() as ctx:
        if reps:
            ctx.enter_context(tc.For_i(0, reps, 1))
        singles = ctx.enter_context(tc.tile_pool(name="singles", bufs=1))
        acts = ctx.enter_context(tc.tile_pool(name="acts", bufs=1))

        # ---- persistent SBUF state ----
        ident = singles.tile([P, P], FP32)
        make_identity(nc, ident)
        cq_sb = singles.tile([P, F // P], FP32)      # [128, 4]
        nc.sync.dma_start(out=cq_sb, in_=cq_d.rearrange("(i p) -> p i", p=P))
        ck_bc = singles.tile([P, F], FP32)
        nc.sync.dma_start(out=ck_bc, in_=_bcast_ap(ck_d[:], P))
        cv_bc = singles.tile([P, F], FP32)
        nc.sync.dma_start(out=cv_bc, in_=_bcast_ap(cv_d[:], P))

        eps_sb = singles.tile([P, 1], FP32)
        nc.vector.memset(eps_sb, EPS)
        v_sb = singles.tile([P, S_TILES, HG, D + 1], FP32R)  # ~33 KB/part
        nc.sync.dma_start(out=v_sb, in_=_r(_bcast_ap(ones_d[:], P)))
        qT_sb = acts.tile([P, PAIRS, S], FP32R)
        kT_sb = acts.tile([P, PAIRS, S], FP32R)
        # per-(pair, q4, half) DRAM bounce rows for the softmax denominators

        # =========== phase 1+2: LN(x), transposes, Q/K/V projections =======
        with tc.tile_pool(name="p12", bufs=2) as p12, \
             tc.tile_pool(name="p12w", bufs=3) as p12w, \
             tc.tile_pool(name="p12s", bufs=8) as p12s, \
             tc.tile_pool(name="ps12", bufs=1, space="PSUM") as ps12:
            for sb in range(SB):
                xnT = p12.tile([P, C_TILES, SBW], FP32R, tag="xnT", bufs=2)
                # ---- x load + LN + transpose into xnT ----
                for t in range(SBW // P):
                    row0 = sb * SBW + t * P
                    x_t = p12.tile([P, C], FP32, tag="x", bufs=3)
                    nc.sync.dma_start(out=x_t, in_=x_d[row0:row0 + P, :])
                    xsum = p12s.tile([P, 1], FP32, tag="st1")
                    xn_t = p12.tile([P, C], FP32, tag="xn", bufs=3)
                    # mean via ACT copy+accum (out is scratch, overwritten below)
                    nc.scalar.activation(xn_t, x_t, AF.Copy, accum_out=xsum)
                    sq_scr = p12.tile([P, C], FP32, tag="sqscr", bufs=2)
                    nc.vector.tensor_mul(sq_scr, x_t, x_t)
                    xsqr = p12s.tile([P, 1], FP32, tag="st2")
                    nc.vector.tensor_reduce(xsqr, sq_scr,
                                            mybir.AxisListType.X, OP.add)
                    xsqm = p12s.tile([P, 1], FP32, tag="st2b")
                    nc.vector.tensor_scalar(xsqm, xsqr, 1.0 / C, None, OP.mult)
                    mu = p12s.tile([P, 1], FP32, tag="st3")
                    nc.vector.tensor_scalar(mu, xsum, 1.0 / C, None, OP.mult)
                    musq = p12s.tile([P, 1], FP32, tag="st4")
                    nc.vector.tensor_mul(musq, mu, mu)
                    var = p12s.tile([P, 1], FP32, tag="st5")
                    nc.vector.tensor_sub(var, xsqm, musq)
                    lnv = p12s.tile([P, 1], FP32, tag="st6")
                    nc.scalar.activation(lnv, var, AF.Ln, bias=eps_sb)
                    rstd = p12s.tile([P, 1], FP32, tag="st7")
                    nc.scalar.activation(rstd, lnv, AF.Exp, scale=-0.5)
                    nmr = p12s.tile([P, 1], FP32, tag="st8")
                    nc.vector.tensor_scalar(nmr, mu, rstd, -1.0, OP.mult, OP.mult)
                    # xn = x*rstd - mu*rstd  (one ACT pass)
                    nc.scalar.activation(xn_t, x_t, AF.Identity,
                                         bias=nmr, scale=rstd)
                    for ci in range(C_TILES):
                        tp = ps12.tile([P, P], FP32, tag="tp", bufs=2)
                        nc.tensor.transpose(tp, xn_t[:, ci * P:(ci + 1) * P],
                                            ident)
                        nc.vector.tensor_copy(xnT[:, ci, t * P:(t + 1) * P], tp)

                # ---- Q projection (transposed out): qT += wq.T @ xnT ----
                psq = [ps12.tile([P, SBW], FP32, tag="proj", bufs=4,
                                 name=f"psq{sb}_{_i}") for _i in range(F // P)]
                for ci in range(C_TILES):
                    w_t = p12w.tile([P, F], FP32R, tag="wstream")
                    nc.sync.dma_start(out=w_t,
                                      in_=_r(wq_d[ci * P:(ci + 1) * P, :]))
                    for fi in range(F // P):
                        nc.tensor.matmul(
                            psq[fi], w_t[:, fi * P:(fi + 1) * P],
                            xnT[:, ci, :],
                            start=(ci == 0), stop=(ci == C_TILES - 1))
                for fi in range(F // P):
                    nc.vector.tensor_scalar(
                        qT_sb[:, fi, sb * SBW:(sb + 1) * SBW], psq[fi],
                        cq_sb[:, fi:fi + 1], None, OP.add)

                # ---- K projection (natural out) + k-LN + transpose ----
                psk = [ps12.tile([P, F], FP32, tag="proj", bufs=4,
                                 name=f"psk{sb}_{_i}") for _i in range(SBW // P)]
                for ci in range(C_TILES):
                    w_t = p12w.tile([P, F], FP32R, tag="wstream")
                    nc.sync.dma_start(out=w_t,
                                      in_=_r(wk_d[ci * P:(ci + 1) * P, :]))
                    for st in range(SBW // P):
                        nc.tensor.matmul(
                            psk[st], xnT[:, ci, st * P:(st + 1) * P],
                            w_t, start=(ci == 0), stop=(ci == C_TILES - 1))
                for st in range(SBW // P):
                    kn = p12.tile([P, HG, D], FP32, tag="kn", bufs=3)
                    nc.vector.tensor_add(kn, psk[st].rearrange(
                        "p (h d) -> p h d", d=D), ck_bc.rearrange(
                        "p (h d) -> p h d", d=D))
                    _ln_hat(nc, tc, p12, p12s, kn, apply_out=kn, eps_sb=eps_sb)
                    # transpose per head pair into kT_sb
                    gst = sb * (SBW // P) + st
                    for pj in range(PAIRS):
                        tpk = ps12.tile([P, P], FP32, tag="tp", bufs=2)
                        nc.tensor.transpose(
                            tpk, kn[:, 2 * pj:2 * pj + 2, :], ident)
                        nc.vector.tensor_copy(
                            kT_sb[:, pj, gst * P:(gst + 1) * P], tpk)

                # ---- V projection (natural out) + v-LN into v_sb ----
                psv = [ps12.tile([P, F], FP32, tag="proj", bufs=4,
                                 name=f"psv{sb}_{_i}") for _i in range(SBW // P)]
                for ci in range(C_TILES):
                    w_t = p12w.tile([P, F], FP32R, tag="wstream")
                    nc.sync.dma_start(out=w_t,
                                      in_=_r(wv_d[ci * P:(ci + 1) * P, :]))
                    for st in range(SBW // P):
                        nc.tensor.matmul(
                            psv[st], xnT[:, ci, st * P:(st + 1) * P],
                            w_t, start=(ci == 0), stop=(ci == C_TILES - 1))
                for st in range(SBW // P):
                    vn = p12.tile([P, HG, D], FP32, tag="vn", bufs=3)
                    nc.vector.tensor_add(vn, psv[st].rearrange(
                        "p (h d) -> p h d", d=D), cv_bc.rearrange(
                        "p (h d) -> p h d", d=D))
                    gst = sb * (SBW // P) + st
                    _ln_hat(nc, tc, p12, p12s, vn,
                            apply_out=v_sb[:, gst, :, 0:D], eps_sb=eps_sb)

        # =========== phase 3: attention per head pair ======================
        attnp = ctx.enter_context(tc.tile_pool(name="attnp", bufs=1))
        with tc.tile_pool(name="p3", bufs=3) as p3, \
             tc.tile_pool(name="p3r", bufs=3) as p3r, \
             tc.tile_pool(name="p3d", bufs=4, space="DRAM") as p3d, \
             tc.tile_pool(name="ps3", bufs=1, space="PSUM") as ps3:
            attnT = attnp.tile([P, PAIRS, S], FP32R)
            wo_sb = attnp.tile([P, F // P, O], FP32R)  # [128, 4, 1024]
            nc.sync.dma_start(out=wo_sb,
                              in_=_r(wo_d.rearrange("(i p) o -> p i o", p=P)))
            for q4 in range(Q4):
                for pj in range(PAIRS):
                    qs = q4 * SBW
                    poA = ps3.tile([D + 1, SBW], FP32, tag="po", bufs=2)
                    poB = ps3.tile([D + 1, SBW], FP32, tag="po", bufs=2)
                    for sk in range(S_TILES):
                        ks = sk * P
                        psAB = ps3.tile([P, 2, SBW], FP32, tag="ps", bufs=2)
                        nc.tensor.matmul(psAB[:, 0, :],
                                         kT_sb[0:D, pj, ks:ks + P],
                                         qT_sb[0:D, pj, qs:qs + SBW])
                        nc.tensor.matmul(psAB[:, 1, :],
                                         kT_sb[D:P, pj, ks:ks + P],
                                         qT_sb[D:P, pj, qs:qs + SBW])
                        eAB = p3.tile([P, 2, SBW], FP32R, tag="e")
                        nc.scalar.activation(eAB, psAB, AF.Exp, scale=SCALE)
                        nc.tensor.matmul(poA, v_sb[:, sk, 2 * pj, :],
                                         eAB[:, 0, :], start=(sk == 0),
                                         stop=(sk == S_TILES - 1))
                        nc.tensor.matmul(poB, v_sb[:, sk, 2 * pj + 1, :],
                                         eAB[:, 1, :], start=(sk == 0),
                                         stop=(sk == S_TILES - 1))
                    for half, po in ((0, poA), (1, poB)):
                        rec = p3r.tile([1, SBW], FP32, tag="rec")
                        nc.vector.reciprocal(rec, po[D:D + 1, :])
                        rdram = p3d.tile([1, SBW], FP32, tag="rd")
                        nc.sync.dma_start(out=rdram, in_=rec)
                        rbc = p3r.tile([D, SBW], FP32, tag="rbc")
                        nc.sync.dma_start(out=rbc,
                                          in_=_bcast_ap(rdram[0, :], D))
                        nc.vector.tensor_mul(
                            attnT[half * D:(half + 1) * D, pj, qs:qs + SBW],
                            po[0:D, :], rbc)

            # ===== phase 4: output projection (overlaps attention tail) ====
            for st in range(S_TILES):
                o_t = p3.tile([P, O], FP32, tag="ot")
                for oi in range(O // SBW):
                    pp = ps3.tile([P, SBW], FP32, tag="pp", bufs=2)
                    for ii in range(F // P):
                        nc.tensor.matmul(
                            pp, attnT[:, ii, st * P:(st + 1) * P],
                            wo_sb[:, ii, oi * SBW:(oi + 1) * SBW],
                            start=(ii == 0), stop=(ii == F // P - 1))
                    nc.vector.tensor_copy(o_t[:, oi * SBW:(oi + 1) * SBW], pp)
                nc.sync.dma_start(out=out_d[st * P:(st + 1) * P, :], in_=o_t)



    return nc


def _ln_hat(nc, tc, pool, spool, t_in, apply_out, eps_sb=None):
    """Per-head layernorm hat: (t - mean_d) * rsqrt(var_d + eps).
    t_in: [P, HG, D] sbuf tile; writes hat into apply_out ([P, HG, D] AP)."""
    sq = pool.tile([P, HG, D], FP32, tag="lnsq", bufs=2)
    nc.vector.tensor_mul(sq, t_in, t_in)
    sm = spool.tile([P, HG], FP32, tag="lns1")
    nc.vector.tensor_reduce(sm, t_in, mybir.AxisListType.X, OP.add)
    sqm = spool.tile([P, HG], FP32, tag="lns2")
    nc.vector.tensor_reduce(sqm, sq, mybir.AxisListType.X, OP.add)
    mu = spool.tile([P, HG], FP32, tag="lns3")
    nc.vector.tensor_scalar(mu, sm, 1.0 / D, None, OP.mult)
    esq = spool.tile([P, HG], FP32, tag="lns4")
    nc.vector.tensor_scalar(esq, sqm, 1.0 / D, None, OP.mult)
    musq = spool.tile([P, HG], FP32, tag="lns5")
    nc.vector.tensor_mul(musq, mu, mu)
    var = spool.tile([P, HG], FP32, tag="lns6")
    nc.vector.tensor_sub(var, esq, musq)
    lnv = spool.tile([P, HG], FP32, tag="lns7")
    nc.scalar.activation(lnv, var, AF.Ln, bias=eps_sb)
    rstd = spool.tile([P, HG], FP32, tag="lns8")
    nc.scalar.activation(rstd, lnv, AF.Exp, scale=-0.5)
    nmr = spool.tile([P, HG], FP32, tag="lns9")
    nc.vector.tensor_mul(nmr, mu, rstd)
    nc.vector.tensor_scalar(nmr, nmr, -1.0, None, OP.mult)
    for h in range(HG):
        nc.scalar.activation(apply_out[:, h, :], t_in[:, h, :], AF.Identity,
                             bias=nmr[:, h:h + 1], scale=rstd[:, h:h + 1])


_NC_CACHE = None


def _get_nc():
    global _NC_CACHE
    if _NC_CACHE is None:
        nc = build_nc()
        split_waits(nc)
        _NC_CACHE = nc
    return _NC_CACHE


def prep_core_inputs(x, norm_g, norm_b, w_qkv, normk_g, normk_b,
                     normv_g, normv_b, w_out, b_out):
    """Host-side fold + shard.  Returns (in_maps, host_bias[core] (O,))."""
    x = np.asarray(x, np.float32)
    norm_g = np.asarray(norm_g, np.float32)
    norm_b = np.asarray(norm_b, np.float32)
    w_qkv = np.asarray(w_qkv, np.float32)
    normk_g = np.asarray(normk_g, np.float32)
    normv_g = np.asarray(normv_g, np.float32)
    normv_b = np.asarray(normv_b, np.float32)
    w_out = np.asarray(w_out, np.float32)

    INNER = HEADS * D
    wq_all, wk_all, wv_all = (w_qkv[:, 0:INNER], w_qkv[:, INNER:2 * INNER],
                              w_qkv[:, 2 * INNER:3 * INNER])
    gk_t = np.tile(normk_g, HG)          # [512] per head-group tiling
    gv_full = np.tile(normv_g, HEADS)
    bv_full = np.tile(normv_b, HEADS)

    in_maps, host_bias = [], []
    for core in range(N_CORES):
        b_idx, hg = divmod(core, 2)
        cols = slice(hg * F, (hg + 1) * F)
        wq = wq_all[:, cols]
        wk = wk_all[:, cols]
        wv = wv_all[:, cols]
        wo = w_out[cols, :]
        # fold norm_g into rows; normk_g into q columns
        wq_f = (norm_g[:, None] * wq) * gk_t[None, :]
        wk_f = norm_g[:, None] * wk
        wv_f = norm_g[:, None] * wv
        cq = (norm_b @ wq) * gk_t
        ck = norm_b @ wk
        cv = norm_b @ wv
        # fold normv_g into w_out rows; normv_b -> host bias
        wo_f = gv_full[cols][:, None] * wo
        host_bias.append(bv_full[cols] @ wo)
        in_maps.append({
            "x_s": np.ascontiguousarray(x[b_idx]),
            "wq": np.ascontiguousarray(wq_f),
            "wk": np.ascontiguousarray(wk_f),
            "wv": np.ascontiguousarray(wv_f),
            "wo": np.ascontiguousarray(wo_f),
            "cq": np.ascontiguousarray(cq),
            "ck": np.ascontiguousarray(ck),
            "cv": np.ascontiguousarray(cv),
            "ones_c": np.ones(S_TILES * HG * (D + 1), np.float32),
        })
    return in_maps, host_bias


def kernel(**inputs):
    nc = _get_nc()
    in_maps, host_bias = prep_core_inputs(**inputs)
    res = run_bass_kernel_spmd(nc, in_maps, list(range(N_CORES)))
    b_out = np.asarray(inputs["b_out"], np.float32)
    out = np.empty((B, S, O), np.float32)
    for b_idx in range(B):
        out[b_idx] = (res.results[2 * b_idx]["out_p"]
                      + res.results[2 * b_idx + 1]["out_p"]
                      + host_bias[2 * b_idx] + host_bias[2 * b_idx + 1]
                      + b_out)
    return out


if __name__ == "__main__":
    nc = build_nc()
    n = sum(len(bb.instructions) for f in nc.m.functions for bb in f.blocks)
    print("built ok,", n, "instructions")


# revision 24
# speedup vs baseline: 1.3253x; 1.0246x over previous
"""Trainium2 Bass kernel for nn_Attention: LN -> QKV -> per-head attention
(with k/v layernorm) -> output projection.

Sharding: 8 cores = 4 batches x 2 head-groups (8 heads each).  Each core
computes its batch's QKV restricted to its heads (no redundant matmul work),
runs attention for its 8 heads, and produces a partial output projection
(contraction over its 512 inner features).  The host sums the two partials
per batch and adds all bias terms.

Host-side weight folds (exact algebra, no device cost):
  - norm_g folded into w_qkv rows;  norm_b @ w_qkv becomes per-feature bias
    vectors cq/ck/cv added on device after each projection.
  - normk_g folded into w_q columns; normk_b dropped (a per-query constant
    added to all scores of a row cancels in softmax).
  - normv_g folded into w_out rows; normv_b term becomes a host-side output
    bias (sum of attention probs is exactly 1).
Softmax is computed without max subtraction (scores are O(1) for LN'd
activations; exp stays well inside fp32 range).  All matmuls run as fp32r.
"""

import os
import sys

import numpy as np

for _p in ("/opt/trn_rl_repo", "/root/.axon_site/_ro/trn_rl_repo"):
    if os.path.isdir(_p) and _p not in sys.path:
        sys.path.append(_p)

import concourse.bass as bass
import concourse.mybir as mybir
import concourse.tile as tile
from concourse.bass_utils import run_bass_kernel_spmd

FP32 = mybir.dt.float32
FP32R = mybir.dt.float32r
AF = mybir.ActivationFunctionType
OP = mybir.AluOpType

B = 4            # batch
S = 2048         # sequence length
C = 1024         # model dim
HEADS = 16
D = 64           # head dim
HG = 8           # heads per core
F = HG * D       # per-core q/k/v feature width (512)
O = 1024         # output dim
P = 128
EPS = 1e-5
N_CORES = 8

S_TILES = S // P          # 16
C_TILES = C // P          # 8
SB = 4                    # seq blocks
SBW = S // SB             # 512 cols per seq block
PAIRS = HG // 2           # 4 head pairs per core
Q4 = 4                    # query blocks of 512
SCALE = D ** -0.5


def _r(t):
    """fp32r view for matmul operands."""
    return t.bitcast(FP32R)


def _bcast_ap(ap_1d, parts):
    """[n] DRAM/SBUF AP -> [parts, n] with 0-step partition broadcast."""
    return bass.AP(tensor=ap_1d.tensor, offset=ap_1d.offset,
                   ap=[[0, parts]] + [list(x) for x in ap_1d.ap])


def _free_bcast(ap2d, n):
    """[p, m] AP -> [p, m, n] broadcasting each element n times along free."""
    return bass.AP(tensor=ap2d.tensor, offset=ap2d.offset,
                   ap=[list(x) for x in ap2d.ap] + [[0, n]])


def split_waits(nc, max_other=1):
    """walrus here rejects >1 sync-wait on TPB_CTRL (Drain) and may reject
    many on others; hoist extra waits onto preceding single-wait NoOps."""
    for f in nc.m.functions:
        for bb in f.blocks:
            new_insts = []
            for inst in bb.instructions:
                si = inst.sync_info
                limit = 1 if isinstance(
                    inst, (mybir.InstDrain, mybir.InstEventSemaphore,
                           mybir.InstNoOp)) else max_other
                if si and si.on_wait and len(si.on_wait) > limit:
                    waits = list(si.on_wait)
                    keep, extra = waits[-limit:], waits[:-limit]
                    for j, w in enumerate(extra):
                        nop = mybir.InstNoOp(
                            name=f"{inst.name}_wsplit_{j}", ins=[], outs=[])
                        nop.engine = inst.engine
                        nop.sync_info = mybir.SyncInfo(on_wait=[w], on_update=[])
                        new_insts.append(nop)
                    inst.sync_info = mybir.SyncInfo(
                        on_wait=keep, on_update=list(si.on_update))
                new_insts.append(inst)
            bb.instructions[:] = new_insts
    return nc


def build_nc(reps=None):
    from contextlib import ExitStack
    from concourse.masks import make_identity

    nc = bass.Bass()
    x_d = nc.declare_dram_parameter("x_s", [S, C], FP32, isOutput=False)
    wq_d = nc.declare_dram_parameter("wq", [C, F], FP32, isOutput=False)
    wk_d = nc.declare_dram_parameter("wk", [C, F], FP32, isOutput=False)
    wv_d = nc.declare_dram_parameter("wv", [C, F], FP32, isOutput=False)
    wo_d = nc.declare_dram_parameter("wo", [F, O], FP32, isOutput=False)
    cq_d = nc.declare_dram_parameter("cq", [F], FP32, isOutput=False)
    ck_d = nc.declare_dram_parameter("ck", [F], FP32, isOutput=False)
    cv_d = nc.declare_dram_parameter("cv", [F], FP32, isOutput=False)
    ones_d = nc.declare_dram_parameter("ones_c", [S_TILES * HG * (D + 1)], FP32, isOutput=False)
    out_d = nc.declare_dram_parameter("out_p", [S, O], FP32, isOutput=True)

    with tile.TileContext(nc) as tc, ExitStack() as ctx:
        if reps:
            ctx.enter_context(tc.For_i(0, reps, 1))
        singles = ctx.enter_context(tc.tile_pool(name="singles", bufs=1))
        acts = ctx.enter_context(tc.tile_pool(name="acts", bufs=1))

        # ---- persistent SBUF state ----
        ident = singles.tile([P, P], FP32)
        make_identity(nc, ident)
        cq_sb = singles.tile([P, F // P], FP32)      # [128, 4]
        nc.sync.dma_start(out=cq_sb, in_=cq_d.rearrange("(i p) -> p i", p=P))
        ck_bc = singles.tile([P, F], FP32)
        nc.sync.dma_start(out=ck_bc, in_=_bcast_ap(ck_d[:], P))
        cv_bc = singles.tile([P, F], FP32)
        nc.sync.dma_start(out=cv_bc, in_=_bcast_ap(cv_d[:], P))

        eps_sb = singles.tile([P, 1], FP32)
        nc.vector.memset(eps_sb, EPS)
        v_sb = singles.tile([P, S_TILES, HG, D + 1], FP32R)  # ~33 KB/part
        nc.sync.dma_start(out=v_sb, in_=_r(_bcast_ap(ones_d[:], P)))
        qT_sb = acts.tile([P, PAIRS, S], FP32R)
        kT_sb = acts.tile([P, PAIRS, S], FP32R)
        # per-(pair, q4, half) DRAM bounce rows for the softmax denominators

        # =========== phase 1+2: LN(x), transposes, Q/K/V projections =======
        with tc.tile_pool(name="p12", bufs=2) as p12, \
             tc.tile_pool(name="p12w", bufs=3) as p12w, \
             tc.tile_pool(name="p12s", bufs=8) as p12s, \
             tc.tile_pool(name="ps12", bufs=1, space="PSUM") as ps12:
            for sb in range(SB):
                xnT = p12.tile([P, C_TILES, SBW], FP32R, tag="xnT", bufs=2)
                # ---- x load + LN + transpose into xnT ----
                for t in range(SBW // P):
                    row0 = sb * SBW + t * P
                    x_t = p12.tile([P, C], FP32, tag="x", bufs=3)
                    nc.sync.dma_start(out=x_t, in_=x_d[row0:row0 + P, :])
                    xsum = p12s.tile([P, 1], FP32, tag="st1")
                    xn_t = p12.tile([P, C], FP32, tag="xn", bufs=3)
                    # mean via ACT copy+accum (out is scratch, overwritten below)
                    nc.scalar.activation(xn_t, x_t, AF.Copy, accum_out=xsum)
                    sq_scr = p12.tile([P, C], FP32, tag="sqscr", bufs=2)
                    nc.vector.tensor_mul(sq_scr, x_t, x_t)
                    xsqr = p12s.tile([P, 1], FP32, tag="st2")
                    nc.vector.tensor_reduce(xsqr, sq_scr,
                                            mybir.AxisListType.X, OP.add)
                    xsqm = p12s.tile([P, 1], FP32, tag="st2b")
                    nc.vector.tensor_scalar(xsqm, xsqr, 1.0 / C, None, OP.mult)
                    mu = p12s.tile([P, 1], FP32, tag="st3")
                    nc.vector.tensor_scalar(mu, xsum, 1.0 / C, None, OP.mult)
                    musq = p12s.tile([P, 1], FP32, tag="st4")
                    nc.vector.tensor_mul(musq, mu, mu)
                    var = p12s.tile([P, 1], FP32, tag="st5")
                    nc.vector.tensor_sub(var, xsqm, musq)
                    lnv = p12s.tile([P, 1], FP32, tag="st6")
                    nc.scalar.activation(lnv, var, AF.Ln, bias=eps_sb)
                    rstd = p12s.tile([P, 1], FP32, tag="st7")
                    nc.scalar.activation(rstd, lnv, AF.Exp, scale=-0.5)
                    nmr = p12s.tile([P, 1], FP32, tag="st8")
                    nc.vector.tensor_scalar(nmr, mu, rstd, -1.0, OP.mult, OP.mult)
                    # xn = x*rstd - mu*rstd  (one ACT pass)
                    nc.scalar.activation(xn_t, x_t, AF.Identity,
                                         bias=nmr, scale=rstd)
                    for ci in range(C_TILES):
                        tp = ps12.tile([P, P], FP32, tag="tp", bufs=2)
                        nc.tensor.transpose(tp, xn_t[:, ci * P:(ci + 1) * P],
                                            ident)
                        dst = xnT[:, ci, t * P:(t + 1) * P]
                        if ci % 2 == 0:
                            nc.vector.tensor_copy(dst, tp)
                        else:
                            nc.scalar.copy(dst, tp)

                # ---- Q projection (transposed out): qT += wq.T @ xnT ----
                psq = [ps12.tile([P, SBW], FP32, tag="proj", bufs=4,
                                 name=f"psq{sb}_{_i}") for _i in range(F // P)]
                for ci in range(C_TILES):
                    w_t = p12w.tile([P, F], FP32R, tag="wstream")
                    nc.sync.dma_start(out=w_t,
                                      in_=_r(wq_d[ci * P:(ci + 1) * P, :]))
                    for fi in range(F // P):
                        nc.tensor.matmul(
                            psq[fi], w_t[:, fi * P:(fi + 1) * P],
                            xnT[:, ci, :],
                            start=(ci == 0), stop=(ci == C_TILES - 1))
                for fi in range(F // P):
                    nc.vector.tensor_scalar(
                        qT_sb[:, fi, sb * SBW:(sb + 1) * SBW], psq[fi],
                        cq_sb[:, fi:fi + 1], None, OP.add)

                # ---- K projection (natural out) + k-LN + transpose ----
                psk = [ps12.tile([P, F], FP32, tag="proj", bufs=4,
                                 name=f"psk{sb}_{_i}") for _i in range(SBW // P)]
                for ci in range(C_TILES):
                    w_t = p12w.tile([P, F], FP32R, tag="wstream")
                    nc.sync.dma_start(out=w_t,
                                      in_=_r(wk_d[ci * P:(ci + 1) * P, :]))
                    for st in range(SBW // P):
                        nc.tensor.matmul(
                            psk[st], xnT[:, ci, st * P:(st + 1) * P],
                            w_t, start=(ci == 0), stop=(ci == C_TILES - 1))
                for st in range(SBW // P):
                    kn = p12.tile([P, HG, D], FP32, tag="kn", bufs=3)
                    nc.vector.tensor_add(kn, psk[st].rearrange(
                        "p (h d) -> p h d", d=D), ck_bc.rearrange(
                        "p (h d) -> p h d", d=D))
                    _ln_hat(nc, tc, p12, p12s, kn, apply_out=kn, eps_sb=eps_sb)
                    # transpose per head pair into kT_sb
                    gst = sb * (SBW // P) + st
                    for pj in range(PAIRS):
                        tpk = ps12.tile([P, P], FP32, tag="tpk", bufs=2)
                        nc.tensor.transpose(
                            tpk, kn[:, 2 * pj:2 * pj + 2, :], ident)
                        if pj % 2 == 0:
                            nc.vector.tensor_copy(
                                kT_sb[:, pj, gst * P:(gst + 1) * P], tpk)
                        else:
                            nc.scalar.copy(
                                kT_sb[:, pj, gst * P:(gst + 1) * P], tpk)

                # ---- V projection (natural out) + v-LN into v_sb ----
                psv = [ps12.tile([P, F], FP32, tag="proj", bufs=4,
                                 name=f"psv{sb}_{_i}") for _i in range(SBW // P)]
                for ci in range(C_TILES):
                    w_t = p12w.tile([P, F], FP32R, tag="wstream")
                    nc.sync.dma_start(out=w_t,
                                      in_=_r(wv_d[ci * P:(ci + 1) * P, :]))
                    for st in range(SBW // P):
                        nc.tensor.matmul(
                            psv[st], xnT[:, ci, st * P:(st + 1) * P],
                            w_t, start=(ci == 0), stop=(ci == C_TILES - 1))
                for st in range(SBW // P):
                    vn = p12.tile([P, HG, D], FP32, tag="vn", bufs=3)
                    nc.vector.tensor_add(vn, psv[st].rearrange(
                        "p (h d) -> p h d", d=D), cv_bc.rearrange(
                        "p (h d) -> p h d", d=D))
                    gst = sb * (SBW // P) + st
                    _ln_hat(nc, tc, p12, p12s, vn,
                            apply_out=v_sb[:, gst, :, 0:D], eps_sb=eps_sb)

        # =========== phase 3: attention per head pair ======================
        attnp = ctx.enter_context(tc.tile_pool(name="attnp", bufs=1))
        with tc.tile_pool(name="p3", bufs=3) as p3, \
             tc.tile_pool(name="p3r", bufs=3) as p3r, \
             tc.tile_pool(name="p3d", bufs=4, space="DRAM") as p3d, \
             tc.tile_pool(name="ps3", bufs=1, space="PSUM") as ps3:
            attnT = attnp.tile([P, PAIRS, S], FP32R)
            wo_sb = attnp.tile([P, F // P, O], FP32R)  # [128, 4, 1024]
            nc.sync.dma_start(out=wo_sb,
                              in_=_r(wo_d.rearrange("(i p) o -> p i o", p=P)))
            for q4 in range(Q4):
                for pj in range(PAIRS):
                    qs = q4 * SBW
                    poA = ps3.tile([D + 1, SBW], FP32, tag="po", bufs=4)
                    poB = ps3.tile([D + 1, SBW], FP32, tag="po", bufs=4)
                    for sk in range(S_TILES):
                        ks = sk * P
                        psAB = ps3.tile([P, 2, SBW], FP32, tag="ps", bufs=2)
                        nc.tensor.matmul(psAB[:, 0, :],
                                         kT_sb[0:D, pj, ks:ks + P],
                                         qT_sb[0:D, pj, qs:qs + SBW])
                        nc.tensor.matmul(psAB[:, 1, :],
                                         kT_sb[D:P, pj, ks:ks + P],
                                         qT_sb[D:P, pj, qs:qs + SBW])
                        eAB = p3.tile([P, 2, SBW], FP32R, tag="e")
                        nc.scalar.activation(eAB, psAB, AF.Exp, scale=SCALE)
                        nc.tensor.matmul(poA, v_sb[:, sk, 2 * pj, :],
                                         eAB[:, 0, :], start=(sk == 0),
                                         stop=(sk == S_TILES - 1))
                        nc.tensor.matmul(poB, v_sb[:, sk, 2 * pj + 1, :],
                                         eAB[:, 1, :], start=(sk == 0),
                                         stop=(sk == S_TILES - 1))
                    for half, po in ((0, poA), (1, poB)):
                        rec = p3r.tile([1, SBW], FP32, tag="rec")
                        nc.vector.reciprocal(rec, po[D:D + 1, :])
                        rdram = p3d.tile([1, SBW], FP32, tag="rd")
                        nc.sync.dma_start(out=rdram, in_=rec)
                        rbc = p3r.tile([D, SBW], FP32, tag="rbc")
                        nc.sync.dma_start(out=rbc,
                                          in_=_bcast_ap(rdram[0, :], D))
                        nc.vector.tensor_mul(
                            attnT[half * D:(half + 1) * D, pj, qs:qs + SBW],
                            po[0:D, :], rbc)

            # ===== phase 4: output projection (overlaps attention tail) ====
            for st in range(S_TILES):
                o_t = p3.tile([P, O], FP32, tag="ot")
                for oi in range(O // SBW):
                    pp = ps3.tile([P, SBW], FP32, tag="po", bufs=4)
                    for ii in range(F // P):
                        nc.tensor.matmul(
                            pp, attnT[:, ii, st * P:(st + 1) * P],
                            wo_sb[:, ii, oi * SBW:(oi + 1) * SBW],
                            start=(ii == 0), stop=(ii == F // P - 1))
                    nc.vector.tensor_copy(o_t[:, oi * SBW:(oi + 1) * SBW], pp)
                nc.sync.dma_start(out=out_d[st * P:(st + 1) * P, :], in_=o_t)



    return nc


def _ln_hat(nc, tc, pool, spool, t_in, apply_out, eps_sb=None):
    """Per-head layernorm hat: (t - mean_d) * rsqrt(var_d + eps).
    t_in: [P, HG, D] sbuf tile; writes hat into apply_out ([P, HG, D] AP)."""
    sq = pool.tile([P, HG, D], FP32, tag="lnsq", bufs=2)
    nc.vector.tensor_mul(sq, t_in, t_in)
    sm = spool.tile([P, HG], FP32, tag="lns1")
    nc.vector.tensor_reduce(sm, t_in, mybir.AxisListType.X, OP.add)
    sqm = spool.tile([P, HG], FP32, tag="lns2")
    nc.vector.tensor_reduce(sqm, sq, mybir.AxisListType.X, OP.add)
    mu = spool.tile([P, HG], FP32, tag="lns3")
    nc.vector.tensor_scalar(mu, sm, 1.0 / D, None, OP.mult)
    esq = spool.tile([P, HG], FP32, tag="lns4")
    nc.vector.tensor_scalar(esq, sqm, 1.0 / D, None, OP.mult)
    musq = spool.tile([P, HG], FP32, tag="lns5")
    nc.vector.tensor_mul(musq, mu, mu)
    var = spool.tile([P, HG], FP32, tag="lns6")
    nc.vector.tensor_sub(var, esq, musq)
    lnv = spool.tile([P, HG], FP32, tag="lns7")
    nc.scalar.activation(lnv, var, AF.Ln, bias=eps_sb)
    rstd = spool.tile([P, HG], FP32, tag="lns8")
    nc.scalar.activation(rstd, lnv, AF.Exp, scale=-0.5)
    nmr = spool.tile([P, HG], FP32, tag="lns9")
    nc.vector.tensor_mul(nmr, mu, rstd)
    nc.vector.tensor_scalar(nmr, nmr, -1.0, None, OP.mult)
    for h in range(HG):
        nc.scalar.activation(apply_out[:, h, :], t_in[:, h, :], AF.Identity,
                             bias=nmr[:, h:h + 1], scale=rstd[:, h:h + 1])


_NC_CACHE = None


def _get_nc():
    global _NC_CACHE
    if _NC_CACHE is None:
        nc = build_nc()
        split_waits(nc)
        _NC_CACHE = nc
    return _NC_CACHE


def prep_core_inputs(x, norm_g, norm_b, w_qkv, normk_g, normk_b,
                     normv_g, normv_b, w_out, b_out):
    """Host-side fold + shard.  Returns (in_maps, host_bias[core] (O,))."""
    x = np.asarray(x, np.float32)
    norm_g = np.asarray(norm_g, np.float32)
    norm_b = np.asarray(norm_b, np.float32)
    w_qkv = np.asarray(w_qkv, np.float32)
    normk_g = np.asarray(normk_g, np.float32)
    normv_g = np.asarray(normv_g, np.float32)
    normv_b = np.asarray(normv_b, np.float32)
    w_out = np.asarray(w_out, np.float32)

    INNER = HEADS * D
    wq_all, wk_all, wv_all = (w_qkv[:, 0:INNER], w_qkv[:, INNER:2 * INNER],
                              w_qkv[:, 2 * INNER:3 * INNER])
    gk_t = np.tile(normk_g, HG)          # [512] per head-group tiling
    gv_full = np.tile(normv_g, HEADS)
    bv_full = np.tile(normv_b, HEADS)

    in_maps, host_bias = [], []
    for core in range(N_CORES):
        b_idx, hg = divmod(core, 2)
        cols = slice(hg * F, (hg + 1) * F)
        wq = wq_all[:, cols]
        wk = wk_all[:, cols]
        wv = wv_all[:, cols]
        wo = w_out[cols, :]
        # fold norm_g into rows; normk_g into q columns
        wq_f = (norm_g[:, None] * wq) * gk_t[None, :]
        wk_f = norm_g[:, None] * wk
        wv_f = norm_g[:, None] * wv
        cq = (norm_b @ wq) * gk_t
        ck = norm_b @ wk
        cv = norm_b @ wv
        # fold normv_g into w_out rows; normv_b -> host bias
        wo_f = gv_full[cols][:, None] * wo
        host_bias.append(bv_full[cols] @ wo)
        in_maps.append({
            "x_s": np.ascontiguousarray(x[b_idx]),
            "wq": np.ascontiguousarray(wq_f),
            "wk": np.ascontiguousarray(wk_f),
            "wv": np.ascontiguousarray(wv_f),
            "wo": np.ascontiguousarray(wo_f),
            "cq": np.ascontiguousarray(cq),
            "ck": np.ascontiguousarray(ck),
            "cv": np.ascontiguousarray(cv),
            "ones_c": np.ones(S_TILES * HG * (D + 1), np.float32),
        })
    return in_maps, host_bias


def kernel(**inputs):
    nc = _get_nc()
    in_maps, host_bias = prep_core_inputs(**inputs)
    res = run_bass_kernel_spmd(nc, in_maps, list(range(N_CORES)))
    b_out = np.asarray(inputs["b_out"], np.float32)
    out = np.empty((B, S, O), np.float32)
    for b_idx in range(B):
        out[b_idx] = (res.results[2 * b_idx]["out_p"]
                      + res.results[2 * b_idx + 1]["out_p"]
                      + host_bias[2 * b_idx] + host_bias[2 * b_idx + 1]
                      + b_out)
    return out


if __name__ == "__main__":
    nc = build_nc()
    n = sum(len(bb.instructions) for f in nc.m.functions for bb in f.blocks)
    print("built ok,", n, "instructions")


# revision 25
# speedup vs baseline: 1.3782x; 1.0399x over previous
"""Trainium2 Bass kernel for nn_Attention: LN -> QKV -> per-head attention
(with k/v layernorm) -> output projection.

Sharding: 8 cores = 4 batches x 2 head-groups (8 heads each).  Each core
computes its batch's QKV restricted to its heads (no redundant matmul work),
runs attention for its 8 heads, and produces a partial output projection
(contraction over its 512 inner features).  The host sums the two partials
per batch and adds all bias terms.

Host-side weight folds (exact algebra, no device cost):
  - norm_g folded into w_qkv rows;  norm_b @ w_qkv becomes per-feature bias
    vectors cq/ck/cv added on device after each projection.
  - normk_g folded into w_q columns; normk_b dropped (a per-query constant
    added to all scores of a row cancels in softmax).
  - normv_g folded into w_out rows; normv_b term becomes a host-side output
    bias (sum of attention probs is exactly 1).
Softmax is computed without max subtraction (scores are O(1) for LN'd
activations; exp stays well inside fp32 range).  All matmuls run as fp32r.
"""

import os
import sys

import numpy as np

for _p in ("/opt/trn_rl_repo", "/root/.axon_site/_ro/trn_rl_repo"):
    if os.path.isdir(_p) and _p not in sys.path:
        sys.path.append(_p)

import concourse.bass as bass
import concourse.mybir as mybir
import concourse.tile as tile
from concourse.bass_utils import run_bass_kernel_spmd

FP32 = mybir.dt.float32
FP32R = mybir.dt.float32r
AF = mybir.ActivationFunctionType
OP = mybir.AluOpType

B = 4            # batch
S = 2048         # sequence length
C = 1024         # model dim
HEADS = 16
D = 64           # head dim
HG = 8           # heads per core
F = HG * D       # per-core q/k/v feature width (512)
O = 1024         # output dim
P = 128
EPS = 1e-5
N_CORES = 8

S_TILES = S // P          # 16
C_TILES = C // P          # 8
SB = 4                    # seq blocks
SBW = S // SB             # 512 cols per seq block
PAIRS = HG // 2           # 4 head pairs per core
Q4 = 4                    # query blocks of 512
SCALE = D ** -0.5


def _r(t):
    """fp32r view for matmul operands."""
    return t.bitcast(FP32R)


def _bcast_ap(ap_1d, parts):
    """[n] DRAM/SBUF AP -> [parts, n] with 0-step partition broadcast."""
    return bass.AP(tensor=ap_1d.tensor, offset=ap_1d.offset,
                   ap=[[0, parts]] + [list(x) for x in ap_1d.ap])


def _free_bcast(ap2d, n):
    """[p, m] AP -> [p, m, n] broadcasting each element n times along free."""
    return bass.AP(tensor=ap2d.tensor, offset=ap2d.offset,
                   ap=[list(x) for x in ap2d.ap] + [[0, n]])


def split_waits(nc, max_other=1):
    """walrus here rejects >1 sync-wait on TPB_CTRL (Drain) and may reject
    many on others; hoist extra waits onto preceding single-wait NoOps."""
    for f in nc.m.functions:
        for bb in f.blocks:
            new_insts = []
            for inst in bb.instructions:
                si = inst.sync_info
                limit = 1 if isinstance(
                    inst, (mybir.InstDrain, mybir.InstEventSemaphore,
                           mybir.InstNoOp)) else max_other
                if si and si.on_wait and len(si.on_wait) > limit:
                    waits = list(si.on_wait)
                    keep, extra = waits[-limit:], waits[:-limit]
                    for j, w in enumerate(extra):
                        nop = mybir.InstNoOp(
                            name=f"{inst.name}_wsplit_{j}", ins=[], outs=[])
                        nop.engine = inst.engine
                        nop.sync_info = mybir.SyncInfo(on_wait=[w], on_update=[])
                        new_insts.append(nop)
                    inst.sync_info = mybir.SyncInfo(
                        on_wait=keep, on_update=list(si.on_update))
                new_insts.append(inst)
            bb.instructions[:] = new_insts
    return nc


def build_nc(reps=None):
    from contextlib import ExitStack
    from concourse.masks import make_identity

    nc = bass.Bass()
    x_d = nc.declare_dram_parameter("x_s", [S, C], FP32, isOutput=False)
    wq_d = nc.declare_dram_parameter("wq", [C, F], FP32, isOutput=False)
    wk_d = nc.declare_dram_parameter("wk", [C, F], FP32, isOutput=False)
    wv_d = nc.declare_dram_parameter("wv", [C, F], FP32, isOutput=False)
    wo_d = nc.declare_dram_parameter("wo", [F, O], FP32, isOutput=False)
    cq_d = nc.declare_dram_parameter("cq", [F], FP32, isOutput=False)
    ck_d = nc.declare_dram_parameter("ck", [F], FP32, isOutput=False)
    cv_d = nc.declare_dram_parameter("cv", [F], FP32, isOutput=False)
    ones_d = nc.declare_dram_parameter("ones_c", [S_TILES * HG * (D + 1)], FP32, isOutput=False)
    out_d = nc.declare_dram_parameter("out_p", [S, O], FP32, isOutput=True)

    with tile.TileContext(nc) as tc, ExitStack() as ctx:
        if reps:
            ctx.enter_context(tc.For_i(0, reps, 1))
        singles = ctx.enter_context(tc.tile_pool(name="singles", bufs=1))
        acts = ctx.enter_context(tc.tile_pool(name="acts", bufs=1))

        # ---- persistent SBUF state ----
        ident = singles.tile([P, P], FP32)
        make_identity(nc, ident)
        cq_sb = singles.tile([P, F // P], FP32)      # [128, 4]
        nc.sync.dma_start(out=cq_sb, in_=cq_d.rearrange("(i p) -> p i", p=P))
        ck_bc = singles.tile([P, F], FP32)
        nc.sync.dma_start(out=ck_bc, in_=_bcast_ap(ck_d[:], P))
        cv_bc = singles.tile([P, F], FP32)
        nc.sync.dma_start(out=cv_bc, in_=_bcast_ap(cv_d[:], P))

        eps_sb = singles.tile([P, 1], FP32)
        nc.vector.memset(eps_sb, EPS)
        v_sb = singles.tile([P, S_TILES, HG, D + 1], FP32R)  # ~33 KB/part
        nc.sync.dma_start(out=v_sb, in_=_r(_bcast_ap(ones_d[:], P)))
        qT_sb = acts.tile([P, PAIRS, S], FP32R)
        kT_sb = acts.tile([P, PAIRS, S], FP32R)
        # per-(pair, q4, half) DRAM bounce rows for the softmax denominators

        # =========== phase 1+2: LN(x), transposes, Q/K/V projections =======
        with tc.tile_pool(name="p12", bufs=2) as p12, \
             tc.tile_pool(name="p12w", bufs=3) as p12w, \
             tc.tile_pool(name="p12s", bufs=8) as p12s, \
             tc.tile_pool(name="ps12", bufs=1, space="PSUM") as ps12:
            for sb in range(SB):
                xnT = p12.tile([P, C_TILES, SBW], FP32R, tag="xnT", bufs=2)
                # ---- x load + LN + transpose into xnT ----
                for t in range(SBW // P):
                    row0 = sb * SBW + t * P
                    x_t = p12.tile([P, C], FP32, tag="x", bufs=3)
                    nc.sync.dma_start(out=x_t, in_=x_d[row0:row0 + P, :])
                    xsum = p12s.tile([P, 1], FP32, tag="st1")
                    xn_t = p12.tile([P, C], FP32, tag="xn", bufs=3)
                    # mean via ACT copy+accum (out is scratch, overwritten below)
                    nc.scalar.activation(xn_t, x_t, AF.Copy, accum_out=xsum)
                    sq_scr = p12.tile([P, C], FP32, tag="sqscr", bufs=2)
                    nc.vector.tensor_mul(sq_scr, x_t, x_t)
                    xsqr = p12s.tile([P, 1], FP32, tag="st2")
                    nc.vector.tensor_reduce(xsqr, sq_scr,
                                            mybir.AxisListType.X, OP.add)
                    xsqm = p12s.tile([P, 1], FP32, tag="st2b")
                    nc.vector.tensor_scalar(xsqm, xsqr, 1.0 / C, None, OP.mult)
                    mu = p12s.tile([P, 1], FP32, tag="st3")
                    nc.vector.tensor_scalar(mu, xsum, 1.0 / C, None, OP.mult)
                    musq = p12s.tile([P, 1], FP32, tag="st4")
                    nc.vector.tensor_mul(musq, mu, mu)
                    var = p12s.tile([P, 1], FP32, tag="st5")
                    nc.vector.tensor_sub(var, xsqm, musq)
                    lnv = p12s.tile([P, 1], FP32, tag="st6")
                    nc.scalar.activation(lnv, var, AF.Ln, bias=eps_sb)
                    rstd = p12s.tile([P, 1], FP32, tag="st7")
                    nc.scalar.activation(rstd, lnv, AF.Exp, scale=-0.5)
                    nmr = p12s.tile([P, 1], FP32, tag="st8")
                    nc.vector.tensor_scalar(nmr, mu, rstd, -1.0, OP.mult, OP.mult)
                    # xn = x*rstd - mu*rstd  (one ACT pass)
                    nc.scalar.activation(xn_t, x_t, AF.Identity,
                                         bias=nmr, scale=rstd)
                    for ci in range(C_TILES):
                        tp = ps12.tile([P, P], FP32, tag="tp", bufs=2)
                        nc.tensor.transpose(tp, xn_t[:, ci * P:(ci + 1) * P],
                                            ident)
                        dst = xnT[:, ci, t * P:(t + 1) * P]
                        if ci % 2 == 0:
                            nc.vector.tensor_copy(dst, tp)
                        else:
                            nc.scalar.copy(dst, tp)

                # ---- Q projection (transposed out): qT += wq.T @ xnT ----
                psq = [ps12.tile([P, SBW], FP32, tag="proj", bufs=4,
                                 name=f"psq{sb}_{_i}") for _i in range(F // P)]
                for ci in range(C_TILES):
                    w_t = p12w.tile([P, F], FP32R, tag="wstream")
                    nc.sync.dma_start(out=w_t,
                                      in_=_r(wq_d[ci * P:(ci + 1) * P, :]))
                    for fi in range(F // P):
                        nc.tensor.matmul(
                            psq[fi], w_t[:, fi * P:(fi + 1) * P],
                            xnT[:, ci, :],
                            start=(ci == 0), stop=(ci == C_TILES - 1))
                for fi in range(F // P):
                    nc.vector.tensor_scalar(
                        qT_sb[:, fi, sb * SBW:(sb + 1) * SBW], psq[fi],
                        cq_sb[:, fi:fi + 1], None, OP.add)

                # ---- K projection (natural out) + k-LN + transpose ----
                psk = [ps12.tile([P, F], FP32, tag="proj", bufs=4,
                                 name=f"psk{sb}_{_i}") for _i in range(SBW // P)]
                for ci in range(C_TILES):
                    w_t = p12w.tile([P, F], FP32R, tag="wstream")
                    nc.sync.dma_start(out=w_t,
                                      in_=_r(wk_d[ci * P:(ci + 1) * P, :]))
                    for st in range(SBW // P):
                        nc.tensor.matmul(
                            psk[st], xnT[:, ci, st * P:(st + 1) * P],
                            w_t, start=(ci == 0), stop=(ci == C_TILES - 1))
                for st in range(SBW // P):
                    kn = p12.tile([P, HG, D], FP32, tag="kn", bufs=3)
                    nc.vector.tensor_add(kn, psk[st].rearrange(
                        "p (h d) -> p h d", d=D), ck_bc.rearrange(
                        "p (h d) -> p h d", d=D))
                    _ln_hat(nc, tc, p12, p12s, kn, apply_out=kn, eps_sb=eps_sb)
                    # transpose per head pair into kT_sb
                    gst = sb * (SBW // P) + st
                    for pj in range(PAIRS):
                        tpk = ps12.tile([P, P], FP32, tag="tpk", bufs=2)
                        nc.tensor.transpose(
                            tpk, kn[:, 2 * pj:2 * pj + 2, :], ident)
                        if pj % 2 == 0:
                            nc.vector.tensor_copy(
                                kT_sb[:, pj, gst * P:(gst + 1) * P], tpk)
                        else:
                            nc.scalar.copy(
                                kT_sb[:, pj, gst * P:(gst + 1) * P], tpk)

                # ---- V projection (natural out) + v-LN into v_sb ----
                psv = [ps12.tile([P, F], FP32, tag="proj", bufs=4,
                                 name=f"psv{sb}_{_i}") for _i in range(SBW // P)]
                for ci in range(C_TILES):
                    w_t = p12w.tile([P, F], FP32R, tag="wstream")
                    nc.sync.dma_start(out=w_t,
                                      in_=_r(wv_d[ci * P:(ci + 1) * P, :]))
                    for st in range(SBW // P):
                        nc.tensor.matmul(
                            psv[st], xnT[:, ci, st * P:(st + 1) * P],
                            w_t, start=(ci == 0), stop=(ci == C_TILES - 1))
                for st in range(SBW // P):
                    vn = p12.tile([P, HG, D], FP32, tag="vn", bufs=3)
                    nc.vector.tensor_add(vn, psv[st].rearrange(
                        "p (h d) -> p h d", d=D), cv_bc.rearrange(
                        "p (h d) -> p h d", d=D))
                    gst = sb * (SBW // P) + st
                    _ln_hat(nc, tc, p12, p12s, vn,
                            apply_out=v_sb[:, gst, :, 0:D], eps_sb=eps_sb)

        # =========== phase 3: attention per head pair ======================
        attnp = ctx.enter_context(tc.tile_pool(name="attnp", bufs=1))
        with tc.tile_pool(name="p3", bufs=4) as p3, \
             tc.tile_pool(name="p3r", bufs=3) as p3r, \
             tc.tile_pool(name="p3d", bufs=4, space="DRAM") as p3d, \
             tc.tile_pool(name="ps3", bufs=1, space="PSUM") as ps3:
            attnT = attnp.tile([P, PAIRS, S], FP32R)
            wo_sb = attnp.tile([P, F // P, O], FP32R)  # [128, 4, 1024]
            nc.sync.dma_start(out=wo_sb,
                              in_=_r(wo_d.rearrange("(i p) o -> p i o", p=P)))
            for q4 in range(Q4):
                for pj in range(PAIRS):
                    qs = q4 * SBW
                    poA = ps3.tile([D + 1, SBW], FP32, tag="po", bufs=4)
                    poB = ps3.tile([D + 1, SBW], FP32, tag="po", bufs=4)
                    for sk in range(S_TILES):
                        ks = sk * P
                        psAB = ps3.tile([P, 2, SBW], FP32, tag="ps", bufs=2)
                        nc.tensor.matmul(psAB[:, 0, :],
                                         kT_sb[0:D, pj, ks:ks + P],
                                         qT_sb[0:D, pj, qs:qs + SBW])
                        nc.tensor.matmul(psAB[:, 1, :],
                                         kT_sb[D:P, pj, ks:ks + P],
                                         qT_sb[D:P, pj, qs:qs + SBW])
                        eAB = p3.tile([P, 2, SBW], FP32R, tag="e")
                        nc.scalar.activation(eAB, psAB, AF.Exp, scale=SCALE)
                        nc.tensor.matmul(poA, v_sb[:, sk, 2 * pj, :],
                                         eAB[:, 0, :], start=(sk == 0),
                                         stop=(sk == S_TILES - 1))
                        nc.tensor.matmul(poB, v_sb[:, sk, 2 * pj + 1, :],
                                         eAB[:, 1, :], start=(sk == 0),
                                         stop=(sk == S_TILES - 1))
                    for half, po in ((0, poA), (1, poB)):
                        rec = p3r.tile([1, SBW], FP32, tag="rec")
                        nc.vector.reciprocal(rec, po[D:D + 1, :])
                        rdram = p3d.tile([1, SBW], FP32, tag="rd")
                        nc.sync.dma_start(out=rdram, in_=rec)
                        rbc = p3r.tile([D, SBW], FP32, tag="rbc")
                        nc.sync.dma_start(out=rbc,
                                          in_=_bcast_ap(rdram[0, :], D))
                        nc.vector.tensor_mul(
                            attnT[half * D:(half + 1) * D, pj, qs:qs + SBW],
                            po[0:D, :], rbc)

            # ===== phase 4: output projection (overlaps attention tail) ====
            for st in range(S_TILES):
                o_t = p3.tile([P, O], FP32, tag="ot")
                for oi in range(O // SBW):
                    pp = ps3.tile([P, SBW], FP32, tag="po", bufs=4)
                    for ii in range(F // P):
                        nc.tensor.matmul(
                            pp, attnT[:, ii, st * P:(st + 1) * P],
                            wo_sb[:, ii, oi * SBW:(oi + 1) * SBW],
                            start=(ii == 0), stop=(ii == F // P - 1))
                    nc.vector.tensor_copy(o_t[:, oi * SBW:(oi + 1) * SBW], pp)
                nc.sync.dma_start(out=out_d[st * P:(st + 1) * P, :], in_=o_t)



    return nc


def _ln_hat(nc, tc, pool, spool, t_in, apply_out, eps_sb=None):
    """Per-head layernorm hat: (t - mean_d) * rsqrt(var_d + eps).
    t_in: [P, HG, D] sbuf tile; writes hat into apply_out ([P, HG, D] AP)."""
    sq = pool.tile([P, HG, D], FP32, tag="lnsq", bufs=2)
    nc.vector.tensor_mul(sq, t_in, t_in)
    sm = spool.tile([P, HG], FP32, tag="lns1")
    nc.vector.tensor_reduce(sm, t_in, mybir.AxisListType.X, OP.add)
    sqm = spool.tile([P, HG], FP32, tag="lns2")
    nc.vector.tensor_reduce(sqm, sq, mybir.AxisListType.X, OP.add)
    mu = spool.tile([P, HG], FP32, tag="lns3")
    nc.vector.tensor_scalar(mu, sm, 1.0 / D, None, OP.mult)
    esq = spool.tile([P, HG], FP32, tag="lns4")
    nc.vector.tensor_scalar(esq, sqm, 1.0 / D, None, OP.mult)
    musq = spool.tile([P, HG], FP32, tag="lns5")
    nc.vector.tensor_mul(musq, mu, mu)
    var = spool.tile([P, HG], FP32, tag="lns6")
    nc.vector.tensor_sub(var, esq, musq)
    lnv = spool.tile([P, HG], FP32, tag="lns7")
    nc.scalar.activation(lnv, var, AF.Ln, bias=eps_sb)
    rstd = spool.tile([P, HG], FP32, tag="lns8")
    nc.scalar.activation(rstd, lnv, AF.Exp, scale=-0.5)
    nmr = spool.tile([P, HG], FP32, tag="lns9")
    nc.vector.tensor_mul(nmr, mu, rstd)
    nc.vector.tensor_scalar(nmr, nmr, -1.0, None, OP.mult)
    for h in range(HG):
        nc.scalar.activation(apply_out[:, h, :], t_in[:, h, :], AF.Identity,
                             bias=nmr[:, h:h + 1], scale=rstd[:, h:h + 1])


_NC_CACHE = None


def _get_nc():
    global _NC_CACHE
    if _NC_CACHE is None:
        nc = build_nc()
        split_waits(nc)
        _NC_CACHE = nc
    return _NC_CACHE


def prep_core_inputs(x, norm_g, norm_b, w_qkv, normk_g, normk_b,
                     normv_g, normv_b, w_out, b_out):
    """Host-side fold + shard.  Returns (in_maps, host_bias[core] (O,))."""
    x = np.asarray(x, np.float32)
    norm_g = np.asarray(norm_g, np.float32)
    norm_b = np.asarray(norm_b, np.float32)
    w_qkv = np.asarray(w_qkv, np.float32)
    normk_g = np.asarray(normk_g, np.float32)
    normv_g = np.asarray(normv_g, np.float32)
    normv_b = np.asarray(normv_b, np.float32)
    w_out = np.asarray(w_out, np.float32)

    INNER = HEADS * D
    wq_all, wk_all, wv_all = (w_qkv[:, 0:INNER], w_qkv[:, INNER:2 * INNER],
                              w_qkv[:, 2 * INNER:3 * INNER])
    gk_t = np.tile(normk_g, HG)          # [512] per head-group tiling
    gv_full = np.tile(normv_g, HEADS)
    bv_full = np.tile(normv_b, HEADS)

    in_maps, host_bias = [], []
    for core in range(N_CORES):
        b_idx, hg = divmod(core, 2)
        cols = slice(hg * F, (hg + 1) * F)
        wq = wq_all[:, cols]
        wk = wk_all[:, cols]
        wv = wv_all[:, cols]
        wo = w_out[cols, :]
        # fold norm_g into rows; normk_g into q columns
        wq_f = (norm_g[:, None] * wq) * gk_t[None, :]
        wk_f = norm_g[:, None] * wk
        wv_f = norm_g[:, None] * wv
        cq = (norm_b @ wq) * gk_t
        ck = norm_b @ wk
        cv = norm_b @ wv
        # fold normv_g into w_out rows; normv_b -> host bias
        wo_f = gv_full[cols][:, None] * wo
        host_bias.append(bv_full[cols] @ wo)
        in_maps.append({
            "x_s": np.ascontiguousarray(x[b_idx]),
            "wq": np.ascontiguousarray(wq_f),
            "wk": np.ascontiguousarray(wk_f),
            "wv": np.ascontiguousarray(wv_f),
            "wo": np.ascontiguousarray(wo_f),
            "cq": np.ascontiguousarray(cq),
            "ck": np.ascontiguousarray(ck),
            "cv": np.ascontiguousarray(cv),
            "ones_c": np.ones(S_TILES * HG * (D + 1), np.float32),
        })
    return in_maps, host_bias


def kernel(**inputs):
    nc = _get_nc()
    in_maps, host_bias = prep_core_inputs(**inputs)
    res = run_bass_kernel_spmd(nc, in_maps, list(range(N_CORES)))
    b_out = np.asarray(inputs["b_out"], np.float32)
    out = np.empty((B, S, O), np.float32)
    for b_idx in range(B):
        out[b_idx] = (res.results[2 * b_idx]["out_p"]
                      + res.results[2 * b_idx + 1]["out_p"]
                      + host_bias[2 * b_idx] + host_bias[2 * b_idx + 1]
                      + b_out)
    return out


if __name__ == "__main__":
    nc = build_nc()
    n = sum(len(bb.instructions) for f in nc.m.functions for bb in f.blocks)
    print("built ok,", n, "instructions")


# revision 27
# speedup vs baseline: 1.6434x; 1.1924x over previous
"""Trainium2 Bass kernel for nn_Attention: LN -> QKV -> per-head attention
(with k/v layernorm) -> output projection.

Sharding: 8 cores = 4 batches x 2 head-groups (8 heads each).  Each core
computes its batch's QKV restricted to its heads (no redundant matmul work),
runs attention for its 8 heads, and produces a partial output projection
(contraction over its 512 inner features).  The host sums the two partials
per batch and adds all bias terms.

Host-side weight folds (exact algebra, no device cost):
  - norm_g folded into w_qkv rows;  norm_b @ w_qkv becomes per-feature bias
    vectors cq/ck/cv added on device after each projection.
  - normk_g folded into w_q columns; normk_b dropped (a per-query constant
    added to all scores of a row cancels in softmax).
  - normv_g folded into w_out rows; normv_b term becomes a host-side output
    bias (sum of attention probs is exactly 1).
Softmax is computed without max subtraction (scores are O(1) for LN'd
activations; exp stays well inside fp32 range).  All matmuls run as fp32r.
"""

import os
import sys

import numpy as np

for _p in ("/opt/trn_rl_repo", "/root/.axon_site/_ro/trn_rl_repo"):
    if os.path.isdir(_p) and _p not in sys.path:
        sys.path.append(_p)

import concourse.bass as bass
import concourse.mybir as mybir
import concourse.tile as tile
from concourse.bass_utils import run_bass_kernel_spmd

FP32 = mybir.dt.float32
FP32R = mybir.dt.float32r
AF = mybir.ActivationFunctionType
OP = mybir.AluOpType

B = 4            # batch
S = 2048         # sequence length
C = 1024         # model dim
HEADS = 16
D = 64           # head dim
HG = 8           # heads per core
F = HG * D       # per-core q/k/v feature width (512)
O = 1024         # output dim
P = 128
EPS = 1e-5
N_CORES = 8

S_TILES = S // P          # 16
C_TILES = C // P          # 8
SB = 4                    # seq blocks
SBW = S // SB             # 512 cols per seq block
PAIRS = HG // 2           # 4 head pairs per core
Q4 = 4                    # query blocks of 512
SCALE = D ** -0.5


def _r(t):
    """fp32r view for matmul operands."""
    return t.bitcast(FP32R)


def _bcast_ap(ap_1d, parts):
    """[n] DRAM/SBUF AP -> [parts, n] with 0-step partition broadcast."""
    return bass.AP(tensor=ap_1d.tensor, offset=ap_1d.offset,
                   ap=[[0, parts]] + [list(x) for x in ap_1d.ap])


def _free_bcast(ap2d, n):
    """[p, m] AP -> [p, m, n] broadcasting each element n times along free."""
    return bass.AP(tensor=ap2d.tensor, offset=ap2d.offset,
                   ap=[list(x) for x in ap2d.ap] + [[0, n]])


def split_waits(nc, max_other=1):
    """walrus here rejects >1 sync-wait on TPB_CTRL (Drain) and may reject
    many on others; hoist extra waits onto preceding single-wait NoOps."""
    for f in nc.m.functions:
        for bb in f.blocks:
            new_insts = []
            for inst in bb.instructions:
                si = inst.sync_info
                limit = 1 if isinstance(
                    inst, (mybir.InstDrain, mybir.InstEventSemaphore,
                           mybir.InstNoOp)) else max_other
                if si and si.on_wait and len(si.on_wait) > limit:
                    waits = list(si.on_wait)
                    keep, extra = waits[-limit:], waits[:-limit]
                    for j, w in enumerate(extra):
                        nop = mybir.InstNoOp(
                            name=f"{inst.name}_wsplit_{j}", ins=[], outs=[])
                        nop.engine = inst.engine
                        nop.sync_info = mybir.SyncInfo(on_wait=[w], on_update=[])
                        new_insts.append(nop)
                    inst.sync_info = mybir.SyncInfo(
                        on_wait=keep, on_update=list(si.on_update))
                new_insts.append(inst)
            bb.instructions[:] = new_insts
    return nc


def build_nc(reps=None):
    from contextlib import ExitStack
    from concourse.masks import make_identity

    nc = bass.Bass()
    x_d = nc.declare_dram_parameter("x_s", [S, C], FP32, isOutput=False)
    wq_d = nc.declare_dram_parameter("wq", [C, F], FP32, isOutput=False)
    wk_d = nc.declare_dram_parameter("wk", [C, F], FP32, isOutput=False)
    wv_d = nc.declare_dram_parameter("wv", [C, F], FP32, isOutput=False)
    wo_d = nc.declare_dram_parameter("wo", [F, O], FP32, isOutput=False)
    cq_d = nc.declare_dram_parameter("cq", [F], FP32, isOutput=False)
    ck_d = nc.declare_dram_parameter("ck", [F], FP32, isOutput=False)
    cv_d = nc.declare_dram_parameter("cv", [F], FP32, isOutput=False)
    ones_d = nc.declare_dram_parameter("ones_c", [S_TILES * HG * (D + 1)], FP32, isOutput=False)
    out_d = nc.declare_dram_parameter("out_p", [S, O], FP32, isOutput=True)

    with tile.TileContext(nc) as tc, ExitStack() as ctx:
        if reps:
            ctx.enter_context(tc.For_i(0, reps, 1))
        singles = ctx.enter_context(tc.tile_pool(name="singles", bufs=1))
        acts = ctx.enter_context(tc.tile_pool(name="acts", bufs=1))

        # ---- persistent SBUF state ----
        ident = singles.tile([P, P], FP32)
        make_identity(nc, ident)
        cq_sb = singles.tile([P, F // P], FP32)      # [128, 4]
        nc.sync.dma_start(out=cq_sb, in_=cq_d.rearrange("(i p) -> p i", p=P))
        ck_bc = singles.tile([P, F], FP32)
        nc.sync.dma_start(out=ck_bc, in_=_bcast_ap(ck_d[:], P))
        cv_bc = singles.tile([P, F], FP32)
        nc.sync.dma_start(out=cv_bc, in_=_bcast_ap(cv_d[:], P))

        eps_sb = singles.tile([P, 1], FP32)
        nc.vector.memset(eps_sb, EPS)
        v_sb = singles.tile([P, S_TILES, HG, D + 1], FP32R)  # ~33 KB/part
        nc.sync.dma_start(out=v_sb, in_=_r(_bcast_ap(ones_d[:], P)))
        qT_sb = acts.tile([P, PAIRS, S], FP32R)
        kT_sb = acts.tile([P, PAIRS, S], FP32R)
        # per-(pair, q4, half) DRAM bounce rows for the softmax denominators

        # =========== phase 1+2: LN(x), transposes, Q/K/V projections =======
        with tc.tile_pool(name="p12", bufs=2) as p12, \
             tc.tile_pool(name="p12w", bufs=3) as p12w, \
             tc.tile_pool(name="p12s", bufs=8) as p12s, \
             tc.tile_pool(name="ps12", bufs=1, space="PSUM") as ps12:
            for sb in range(SB):
                xnT = p12.tile([P, C_TILES, SBW], FP32R, tag="xnT", bufs=2)
                # ---- x load + LN + transpose into xnT ----
                for t in range(SBW // P):
                    row0 = sb * SBW + t * P
                    x_t = p12.tile([P, C], FP32, tag="x", bufs=3)
                    nc.sync.dma_start(out=x_t, in_=x_d[row0:row0 + P, :])
                    xsum = p12s.tile([P, 1], FP32, tag="st1")
                    xn_t = p12.tile([P, C], FP32, tag="xn", bufs=3)
                    # mean via ACT copy+accum (out is scratch, overwritten below)
                    nc.scalar.activation(xn_t, x_t, AF.Copy, accum_out=xsum)
                    sq_scr = p12.tile([P, C], FP32, tag="sqscr", bufs=2)
                    nc.vector.tensor_mul(sq_scr, x_t, x_t)
                    xsqr = p12s.tile([P, 1], FP32, tag="st2")
                    nc.vector.tensor_reduce(xsqr, sq_scr,
                                            mybir.AxisListType.X, OP.add)
                    xsqm = p12s.tile([P, 1], FP32, tag="st2b")
                    nc.vector.tensor_scalar(xsqm, xsqr, 1.0 / C, None, OP.mult)
                    mu = p12s.tile([P, 1], FP32, tag="st3")
                    nc.vector.tensor_scalar(mu, xsum, 1.0 / C, None, OP.mult)
                    musq = p12s.tile([P, 1], FP32, tag="st4")
                    nc.vector.tensor_mul(musq, mu, mu)
                    var = p12s.tile([P, 1], FP32, tag="st5")
                    nc.vector.tensor_sub(var, xsqm, musq)
                    lnv = p12s.tile([P, 1], FP32, tag="st6")
                    nc.scalar.activation(lnv, var, AF.Ln, bias=eps_sb)
                    rstd = p12s.tile([P, 1], FP32, tag="st7")
                    nc.scalar.activation(rstd, lnv, AF.Exp, scale=-0.5)
                    nmr = p12s.tile([P, 1], FP32, tag="st8")
                    nc.vector.tensor_scalar(nmr, mu, rstd, -1.0, OP.mult, OP.mult)
                    # xn = x*rstd - mu*rstd  (one ACT pass)
                    nc.scalar.activation(xn_t, x_t, AF.Identity,
                                         bias=nmr, scale=rstd)
                    for ci in range(C_TILES):
                        tp = ps12.tile([P, P], FP32, tag="tp", bufs=2)
                        nc.tensor.transpose(tp, xn_t[:, ci * P:(ci + 1) * P],
                                            ident)
                        dst = xnT[:, ci, t * P:(t + 1) * P]
                        if ci % 2 == 0:
                            nc.vector.tensor_copy(dst, tp)
                        else:
                            nc.scalar.copy(dst, tp)

                # ---- Q projection (transposed out): qT += wq.T @ xnT ----
                psq = [ps12.tile([P, SBW], FP32, tag="proj", bufs=4,
                                 name=f"psq{sb}_{_i}") for _i in range(F // P)]
                for ci in range(C_TILES):
                    w_t = p12w.tile([P, F], FP32R, tag="wstream")
                    nc.sync.dma_start(out=w_t,
                                      in_=_r(wq_d[ci * P:(ci + 1) * P, :]))
                    for fi in range(F // P):
                        nc.tensor.matmul(
                            psq[fi], w_t[:, fi * P:(fi + 1) * P],
                            xnT[:, ci, :],
                            start=(ci == 0), stop=(ci == C_TILES - 1))
                for fi in range(F // P):
                    nc.vector.tensor_scalar(
                        qT_sb[:, fi, sb * SBW:(sb + 1) * SBW], psq[fi],
                        cq_sb[:, fi:fi + 1], None, OP.add)

                # ---- K projection (natural out) + k-LN + transpose ----
                psk = [ps12.tile([P, F], FP32, tag="proj", bufs=4,
                                 name=f"psk{sb}_{_i}") for _i in range(SBW // P)]
                for ci in range(C_TILES):
                    w_t = p12w.tile([P, F], FP32R, tag="wstream")
                    nc.sync.dma_start(out=w_t,
                                      in_=_r(wk_d[ci * P:(ci + 1) * P, :]))
                    for st in range(SBW // P):
                        nc.tensor.matmul(
                            psk[st], xnT[:, ci, st * P:(st + 1) * P],
                            w_t, start=(ci == 0), stop=(ci == C_TILES - 1))
                for st in range(SBW // P):
                    kn = p12.tile([P, HG, D], FP32, tag="kn", bufs=3)
                    nc.vector.tensor_add(kn, psk[st].rearrange(
                        "p (h d) -> p h d", d=D), ck_bc.rearrange(
                        "p (h d) -> p h d", d=D))
                    _ln_hat(nc, tc, p12, p12s, kn, apply_out=kn, eps_sb=eps_sb)
                    # transpose per head pair into kT_sb
                    gst = sb * (SBW // P) + st
                    for pj in range(PAIRS):
                        tpk = ps12.tile([P, P], FP32, tag="tpk", bufs=2)
                        nc.tensor.transpose(
                            tpk, kn[:, 2 * pj:2 * pj + 2, :], ident)
                        if pj % 2 == 0:
                            nc.vector.tensor_copy(
                                kT_sb[:, pj, gst * P:(gst + 1) * P], tpk)
                        else:
                            nc.scalar.copy(
                                kT_sb[:, pj, gst * P:(gst + 1) * P], tpk)

                # ---- V projection (natural out) + v-LN into v_sb ----
                psv = [ps12.tile([P, F], FP32, tag="proj", bufs=4,
                                 name=f"psv{sb}_{_i}") for _i in range(SBW // P)]
                for ci in range(C_TILES):
                    w_t = p12w.tile([P, F], FP32R, tag="wstream")
                    nc.sync.dma_start(out=w_t,
                                      in_=_r(wv_d[ci * P:(ci + 1) * P, :]))
                    for st in range(SBW // P):
                        nc.tensor.matmul(
                            psv[st], xnT[:, ci, st * P:(st + 1) * P],
                            w_t, start=(ci == 0), stop=(ci == C_TILES - 1))
                for st in range(SBW // P):
                    vn = p12.tile([P, HG, D], FP32, tag="vn", bufs=3)
                    nc.vector.tensor_add(vn, psv[st].rearrange(
                        "p (h d) -> p h d", d=D), cv_bc.rearrange(
                        "p (h d) -> p h d", d=D))
                    gst = sb * (SBW // P) + st
                    _ln_hat(nc, tc, p12, p12s, vn,
                            apply_out=v_sb[:, gst, :, 0:D], eps_sb=eps_sb)

        # =========== phase 3: attention per head pair ======================
        attnp = ctx.enter_context(tc.tile_pool(name="attnp", bufs=1))
        with tc.tile_pool(name="p3", bufs=4) as p3, \
             tc.tile_pool(name="p3r", bufs=3) as p3r, \
             tc.tile_pool(name="p3d", bufs=4, space="DRAM") as p3d, \
             tc.tile_pool(name="ps3", bufs=1, space="PSUM") as ps3:
            attnT = attnp.tile([P, PAIRS, S], FP32R)
            wo_sb = attnp.tile([P, F // P, O], FP32R)  # [128, 4, 1024]
            nc.sync.dma_start(out=wo_sb,
                              in_=_r(wo_d.rearrange("(i p) o -> p i o", p=P)))
            for q4 in range(Q4):
                for pj in range(PAIRS):
                    qs = q4 * SBW
                    poA = ps3.tile([D + 1, SBW], FP32, tag="po", bufs=4)
                    poB = ps3.tile([D + 1, SBW], FP32, tag="po", bufs=4)
                    for sk in range(S_TILES):
                        ks = sk * P
                        psAB = ps3.tile([P, 2, SBW], FP32, tag="ps", bufs=2)
                        nc.tensor.matmul(psAB[:, 0, :],
                                         kT_sb[0:D, pj, ks:ks + P],
                                         qT_sb[0:D, pj, qs:qs + SBW])
                        nc.tensor.matmul(psAB[:, 1, :],
                                         kT_sb[D:P, pj, ks:ks + P],
                                         qT_sb[D:P, pj, qs:qs + SBW])
                        eAB = p3.tile([P, 2, SBW], FP32R, tag="e")
                        nc.scalar.activation(eAB, psAB, AF.Exp, scale=SCALE)
                        nc.tensor.matmul(poA, v_sb[:, sk, 2 * pj, :],
                                         eAB[:, 0, :], start=(sk == 0),
                                         stop=(sk == S_TILES - 1))
                        nc.tensor.matmul(poB, v_sb[:, sk, 2 * pj + 1, :],
                                         eAB[:, 1, :], start=(sk == 0),
                                         stop=(sk == S_TILES - 1))
                    for half, po in ((0, poA), (1, poB)):
                        rec = p3r.tile([1, SBW], FP32, tag="rec")
                        nc.vector.reciprocal(rec, po[D:D + 1, :])
                        rdram = p3d.tile([1, SBW], FP32, tag="rd")
                        nc.sync.dma_start(out=rdram, in_=rec)
                        rbc = p3r.tile([D, SBW], FP32, tag="rbc")
                        nc.sync.dma_start(out=rbc,
                                          in_=_bcast_ap(rdram[0, :], D))
                        nc.vector.tensor_mul(
                            attnT[half * D:(half + 1) * D, pj, qs:qs + SBW],
                            po[0:D, :], rbc)

            # ===== phase 4: output projection (overlaps attention tail) ====
            for st in range(S_TILES):
                o_t = p3.tile([P, O], FP32, tag="ot")
                for oi in range(O // SBW):
                    pp = ps3.tile([P, SBW], FP32, tag="po", bufs=4)
                    for ii in range(F // P):
                        nc.tensor.matmul(
                            pp, attnT[:, ii, st * P:(st + 1) * P],
                            wo_sb[:, ii, oi * SBW:(oi + 1) * SBW],
                            start=(ii == 0), stop=(ii == F // P - 1))
                    nc.vector.tensor_copy(o_t[:, oi * SBW:(oi + 1) * SBW], pp)
                nc.sync.dma_start(out=out_d[st * P:(st + 1) * P, :], in_=o_t)



    return nc


def _ln_hat(nc, tc, pool, spool, t_in, apply_out, eps_sb=None):
    """Per-head layernorm hat: (t - mean_d) * rsqrt(var_d + eps).
    t_in: [P, HG, D] sbuf tile; writes hat into apply_out ([P, HG, D] AP)."""
    sq = pool.tile([P, HG, D], FP32, tag="lnsq", bufs=2)
    nc.vector.tensor_mul(sq, t_in, t_in)
    sm = spool.tile([P, HG], FP32, tag="lns1")
    nc.vector.tensor_reduce(sm, t_in, mybir.AxisListType.X, OP.add)
    sqm = spool.tile([P, HG], FP32, tag="lns2")
    nc.vector.tensor_reduce(sqm, sq, mybir.AxisListType.X, OP.add)
    mu = spool.tile([P, HG], FP32, tag="lns3")
    nc.vector.tensor_scalar(mu, sm, 1.0 / D, None, OP.mult)
    esq = spool.tile([P, HG], FP32, tag="lns4")
    nc.vector.tensor_scalar(esq, sqm, 1.0 / D, None, OP.mult)
    musq = spool.tile([P, HG], FP32, tag="lns5")
    nc.vector.tensor_mul(musq, mu, mu)
    var = spool.tile([P, HG], FP32, tag="lns6")
    nc.vector.tensor_sub(var, esq, musq)
    lnv = spool.tile([P, HG], FP32, tag="lns7")
    nc.scalar.activation(lnv, var, AF.Ln, bias=eps_sb)
    rstd = spool.tile([P, HG], FP32, tag="lns8")
    nc.scalar.activation(rstd, lnv, AF.Exp, scale=-0.5)
    nmr = spool.tile([P, HG], FP32, tag="lns9")
    nc.vector.tensor_mul(nmr, mu, rstd)
    nc.vector.tensor_scalar(nmr, nmr, -1.0, None, OP.mult)
    for h in range(HG):
        nc.scalar.activation(apply_out[:, h, :], t_in[:, h, :], AF.Identity,
                             bias=nmr[:, h:h + 1], scale=rstd[:, h:h + 1])


_NC_CACHE = None


def _get_nc():
    global _NC_CACHE
    if _NC_CACHE is None:
        nc = build_nc()
        split_waits(nc)
        _NC_CACHE = nc
    return _NC_CACHE


def prep_core_inputs(x, norm_g, norm_b, w_qkv, normk_g, normk_b,
                     normv_g, normv_b, w_out, b_out):
    """Host-side fold + shard.  Returns (in_maps, host_bias[core] (O,))."""
    x = np.asarray(x, np.float32)
    norm_g = np.asarray(norm_g, np.float32)
    norm_b = np.asarray(norm_b, np.float32)
    w_qkv = np.asarray(w_qkv, np.float32)
    normk_g = np.asarray(normk_g, np.float32)
    normv_g = np.asarray(normv_g, np.float32)
    normv_b = np.asarray(normv_b, np.float32)
    w_out = np.asarray(w_out, np.float32)

    INNER = HEADS * D
    wq_all, wk_all, wv_all = (w_qkv[:, 0:INNER], w_qkv[:, INNER:2 * INNER],
                              w_qkv[:, 2 * INNER:3 * INNER])
    gk_t = np.tile(normk_g, HG)          # [512] per head-group tiling
    gv_full = np.tile(normv_g, HEADS)
    bv_full = np.tile(normv_b, HEADS)

    in_maps, host_bias = [], []
    for core in range(N_CORES):
        b_idx, hg = divmod(core, 2)
        cols = slice(hg * F, (hg + 1) * F)
        wq = wq_all[:, cols]
        wk = wk_all[:, cols]
        wv = wv_all[:, cols]
        wo = w_out[cols, :]
        # fold norm_g into rows; normk_g into q columns
        wq_f = (norm_g[:, None] * wq) * gk_t[None, :]
        wk_f = norm_g[:, None] * wk
        wv_f = norm_g[:, None] * wv
        cq = (norm_b @ wq) * gk_t
        ck = norm_b @ wk
        cv = norm_b @ wv
        # fold normv_g into w_out rows; normv_b -> host bias
        wo_f = gv_full[cols][:, None] * wo
        host_bias.append(bv_full[cols] @ wo)
        in_maps.append({
            "x_s": np.ascontiguousarray(x[b_idx]),
            "wq": np.ascontiguousarray(wq_f),
            "wk": np.ascontiguousarray(wk_f),
            "wv": np.ascontiguousarray(wv_f),
            "wo": np.ascontiguousarray(wo_f),
            "cq": np.ascontiguousarray(cq),
            "ck": np.ascontiguousarray(ck),
            "cv": np.ascontiguousarray(cv),
            "ones_c": np.ones(S_TILES * HG * (D + 1), np.float32),
        })
    return in_maps, host_bias


def kernel(**inputs):
    nc = _get_nc()
    in_maps, host_bias = prep_core_inputs(**inputs)
    res = run_bass_kernel_spmd(nc, in_maps, list(range(N_CORES)))
    b_out = np.asarray(inputs["b_out"], np.float32)
    out = np.empty((B, S, O), np.float32)
    for b_idx in range(B):
        out[b_idx] = (res.results[2 * b_idx]["out_p"]
                      + res.results[2 * b_idx + 1]["out_p"]
                      + host_bias[2 * b_idx] + host_bias[2 * b_idx + 1]
                      + b_out)
    return out


if __name__ == "__main__":
    nc = build_nc()
    n = sum(len(bb.instructions) for f in nc.m.functions for bb in f.blocks)
    print("built ok,", n, "instructions")
